# revision 8
# baseline (speedup 1.0000x reference)
"""Trainium2 Bass kernel v3 for nn_MfdFC. See kernel_v2 docstring for math.

v3 over v2:
- host pre-transposes x into per-(block,half) [i, j, d] layout and w into the
  replicated [128, 1024] SBUF image -> all input DMAs are contiguous; the
  output is written in SBUF-natural [o, j, d] order and re-transposed on host.
- the per-block [128,16] "smalls" pipelines (iter-0 f-chain, update-phase
  cos/sin/rsqrt chain) run once per interleave-PAIR on [128,32] tiles, and
  the coefA/|G|^2 reductions of a pair are fused into ONE 4096-wide reduce.
- optional GPSIMD offload for selected elementwise passes (t2, prod).
"""
import math
import numpy as np

f32 = np.float32

C_IN = 64
C_OUT = 64
D_DIM = 64
ROWS_PER_CORE = 128
N_CORES = 8
R = 16
NBLK = 4
W = 64 * R
RSQ_C1 = 1.7584694439735017e-30
RSQ_C2 = -2.755803843779718e-20
HALF_PI = float(f32(math.pi / 2.0))
EPS_U = float(f32(2.0 ** -22))

_COMPILED = {}

def _register_custom_ops():
    import concourse.dve_ops as dve_ops
    from concourse.dve_ops import DveOp
    from concourse.dve_spec import (
        Spec, Src0, Src1, C0, C1, lower, maxx, _has_src1 as has_src1,
    )
    from concourse.dve_uop import DveOpSpec
    from concourse.dve_table_gen import dve_ver_for

    if "ANT_RSQ_F" in dve_ops._SUB_OPCODE_FOR_NAME:
        return {n: op for n, op in ((o.name, o) for o in dve_ops.OPS)
                if n.startswith("ANT_")}

    def _ref_rsq_f(in0, in1, s0, s1, imm2):
        u = np.asarray(in0, f32)
        nt = np.asarray(in1, f32)
        m1 = (nt * f32(s0)).astype(f32)
        m2 = (m1 * nt).astype(f32)
        m3 = (m2 * f32(s0)).astype(f32)
        t = (m3 * u).astype(f32)
        return ((t + f32(s1)) * nt).astype(f32)

    _m1 = Src1 * C0
    _m3 = (_m1 * Src1) * C0
    RSQ_F = DveOp("ANT_RSQ_F",
                  Spec(body=((_m3 * Src0) + C1) * Src1, reference=_ref_rsq_f),
                  subdim=False, uops_sha={})

    def _ref_rsq_nr(in0, in1, s0, s1, imm2):
        u = np.asarray(in0, f32); y = np.asarray(in1, f32)
        a = (u * y).astype(f32)
        b = (a * y).astype(f32)
        return ((f32(s0) - (b * f32(s1)).astype(f32)) * y).astype(f32)

    RSQ_NR = DveOp("ANT_RSQ_NR",
                   Spec(body=(C0 - ((Src0 * Src1) * Src1) * C1) * Src1,
                        reference=_ref_rsq_nr),
                   subdim=False, uops_sha={})

    def _ref_gn2(in0, in1, s0, s1, imm2):
        raw = np.asarray(in0, f32); c = np.asarray(in1, f32)
        return np.maximum((raw - (c * c).astype(f32)).astype(f32), f32(s0))

    GN2_F = DveOp("ANT_GN2_F",
                  Spec(body=maxx(Src0 - Src1 * Src1, C0), reference=_ref_gn2),
                  subdim=False, uops_sha={})

    ops = [RSQ_F, RSQ_NR, GN2_F]
    base = dve_ops._CUSTOM_DVE_ROW_BASE + len(dve_ops.OPS)
    for i, op in enumerate(ops):
        dve_ops._SUB_OPCODE_FOR_NAME[op.name] = base + i
    for trn in ("TRN2",):
        ver = dve_ver_for(trn)
        for op in ops:
            uops = lower(op.spec, ver=ver)
            s = DveOpSpec(name=op.name, opcode=dve_ops.get_dve_sub_opcode(op.name),
                          uops=uops, rd1_en=has_src1(op.spec))
            op.uops_sha[ver] = s.sha(ver)
    dve_ops.OPS.extend(ops)
    dve_ops.CUSTOM_DVE_SPECS.update({op.name: op.spec for op in ops})
    return {op.name: op for op in ops}



def _ensure_trace_hook():
    try:
        from antenv.axon_hooks import get_axon_ntff_profile_hook
        return get_axon_ntff_profile_hook() is not None
    except ImportError:
        pass
    try:
        import sys, types
        import antenv
        from trn_agent_boot.trn_boot import _ntff_profile_via_ctypes
        mod = types.ModuleType("antenv.axon_hooks")
        _h = {}
        mod.set_axon_ntff_profile_hook = lambda h: _h.__setitem__("h", h)
        mod.get_axon_ntff_profile_hook = lambda: _h.get("h")
        sys.modules["antenv.axon_hooks"] = mod
        antenv.axon_hooks = mod
        mod.set_axon_ntff_profile_hook(
            _ntff_profile_via_ctypes("/opt/axon/libaxon_pjrt.so"))
        return True
    except Exception:
        return False





def build_program(INTERLEAVE=4, gps=(), redsplit=False, wbufs=2,
                  stagger=False, psf=2, pst=2, pss=2, dbufs=None, fold2=True, fold3=False, dup=True, qdve=False, g2dve=False, cpdve=False, smaj=False):
    from contextlib import ExitStack
    import concourse.bacc as bacc
    import concourse.mybir as mybir
    import concourse.tile as tile

    gps = frozenset(gps)
    FP = mybir.dt.float32
    BF = mybir.dt.bfloat16
    I32 = mybir.dt.int32
    AF = mybir.ActivationFunctionType
    ALU = mybir.AluOpType
    AX = mybir.AxisListType

    OPS = _register_custom_ops()
    RSQ_F, RSQ_NR, GN2_F = OPS["ANT_RSQ_F"], OPS["ANT_RSQ_NR"], OPS["ANT_GN2_F"]

    nc = bacc.Bacc()
    # x pre-transposed on host: [block, half, i, j, d]
    x_d = nc.dram_tensor("xp", (NBLK, 2, C_IN, R, D_DIM), BF,
                         kind="ExternalInput")
    # x0 rows: [block, half, j, d]
    x0_d = nc.dram_tensor("x0p", (NBLK, 2, R, D_DIM), BF, kind="ExternalInput")
    w_d = nc.dram_tensor("w_rep", (128, W), BF, kind="ExternalInput")
    id_d = nc.dram_tensor("ident2", (128, 64), BF, kind="ExternalInput")
    # output in SBUF-natural order: [block, half, o, j, d]
    out_d = nc.dram_tensor("out_p", (NBLK, 2, C_OUT, R, D_DIM), BF,
                           kind="ExternalOutput")

    ctx = ExitStack()
    with ctx:
        tc = ctx.enter_context(tile.TileContext(nc))
        const = ctx.enter_context(tc.tile_pool(name="const", bufs=1))
        xg_p = ctx.enter_context(tc.tile_pool(name="xg", bufs=NBLK))
        work = ctx.enter_context(tc.tile_pool(name="work", bufs=wbufs))
        deep = ctx.enter_context(tc.tile_pool(name="deep", bufs=dbufs or max(2, INTERLEAVE)))
        ab_p = ctx.enter_context(tc.tile_pool(name="ab", bufs=NBLK))
        red_p = ctx.enter_context(tc.tile_pool(name="red", bufs=max(2, INTERLEAVE)))
        ps_f = ctx.enter_context(tc.tile_pool(name="psf", bufs=psf, space="PSUM"))
        ps_t = ctx.enter_context(tc.tile_pool(name="pst", bufs=pst, space="PSUM"))
        ps_s = ctx.enter_context(tc.tile_pool(name="pss", bufs=pss, space="PSUM"))

        def eng(name):
            return nc.gpsimd if name in gps else nc.vector

        # ---- constants (all contiguous DMAs)
        w_g = const.tile([128, W], BF, tag="wg")
        nc.sync.dma_start(w_g[:, :], w_d[:, :])
        ident = const.tile([128, 64], BF, tag="ident")
        nc.sync.dma_start(ident[:, :], id_d[:, :])
        halfpi = const.tile([128, 1], FP, tag="halfpi")
        nc.vector.memset(halfpi[:, :], HALF_PI)

        def jbh(t, h, j):
            return t[64 * h:64 * h + 64, 64 * j:64 * j + 64]

        def b3(t):
            return t[:, :].rearrange("p (j d) -> p j d", d=64)

        def bcR(small_ap):      # [128, R] ap -> broadcast (p, j, 64)
            return small_ap.rearrange("p (j o) -> p j o", o=1) \
                .broadcast_to([128, R, 64])

        def emit_load(st):
            b = st["b"]
            X = xg_p.tile([128, W], BF, tag="xg")
            for h in (0, 1):
                nc.sync.dma_start(b3(X[64 * h:64 * h + 64, :]), x_d[b, h])
            A0 = ab_p.tile([128, W], BF, tag="a0")
            for h in (0, 1):
                nc.sync.dma_start(
                    A0[64 * h:64 * h + 64, :].rearrange("p (j d) -> p j d", d=64),
                    x0_d[b:b + 1, h].rearrange("b j d -> b j d")
                    .broadcast_to([64, R, 64]))
            tp = ps_t.tile([128, W], BF, tag="tp")
            for h in (0, 1):
                for r in range(R):
                    nc.tensor.transpose(jbh(tp, h, r), jbh(X, h, r),
                                        ident[64 * h:64 * h + 64, :])
            XT = xg_p.tile([128, W], BF, tag="xt")
            (nc.vector.tensor_copy if cpdve else nc.scalar.copy)(
                XT[:, :], tp[:, :])
            st["X"], st["XT"], st["A"] = X, XT, A0

        def quake(pool, src_ap, shape, tagp, out_dt, nr=False):
            seed = pool.tile(shape, FP, tag=tagp + "sd")
            nc.vector.tensor_scalar(seed[:, :].bitcast(I32),
                                    src_ap.bitcast(I32), 1, -1,
                                    ALU.logical_shift_right, ALU.bitwise_xor)
            rr = pool.tile(shape, out_dt, tag=tagp + "rr")
            nc.vector._custom_dve(RSQ_F, out=rr[:, :], in0=src_ap,
                                  in1=seed[:, :], s0=RSQ_C1, s1=RSQ_C2)
            if not nr:
                return rr
            rr2 = pool.tile(shape, out_dt, tag=tagp + "r2")
            nc.vector._custom_dve(RSQ_NR, out=rr2[:, :], in0=src_ap,
                                  in1=rr[:, :], s0=1.5, s1=0.5)
            return rr2

        # ---------- iteration 0: per-block D0 + small f-chain
        def emit_d0(st):
            XT = st["XT"]
            psD0 = ps_s.tile([128, R], FP, tag="d0")
            for h in (0, 1):
                for r in range(R):
                    nc.tensor.matmul(
                        psD0[64 * h:64 * h + 64, r:r + 1],
                        jbh(XT, h, r),
                        XT[64 * h:64 * h + 64, 64 * r:64 * r + 1])
            shape = [128, R]
            q0 = red_p.tile(shape, FP, tag="f0q")
            nc.scalar.activation(q0[:, :], psD0[:, :], AF.Square)
            u0 = red_p.tile(shape, FP, tag="f0u")
            nc.vector.tensor_scalar(u0[:, :], q0[:, :], -1.0, 1.0 + EPS_U,
                                    ALU.mult, ALU.add)
            rr0 = quake(red_p, u0[:, :], shape, "f0", FP)
            zs0 = red_p.tile(shape, FP, tag="f0z")
            nc.vector.tensor_tensor(zs0[:, :], psD0[:, :], rr0[:, :], ALU.mult)
            th0 = red_p.tile(shape, FP, tag="f0t")
            nc.scalar.activation(th0[:, :], zs0[:, :], AF.Arctan, scale=-1.0)
            f0 = red_p.tile(shape, BF, tag="f0v")
            nc.vector.scalar_tensor_tensor(f0[:, :], th0[:, :], HALF_PI,
                                           rr0[:, :], ALU.add, ALU.mult)
            st["f0"] = f0[:, :]

        def emit_factor(st, it):
            X, XT = st["X"], st["XT"]
            if it == 0:
                Xf = deep.tile([128, W], BF, tag="xf")
                nc.vector.tensor_tensor(b3(Xf), b3(X), bcR(st["f0"]), ALU.mult)
                st["Xf"] = Xf
                return
            AT = st["AT"]
            psD = ps_f.tile([128, W], FP, tag="mmf")
            for h in (0, 1):
                for r in range(R):
                    nc.tensor.matmul(jbh(psD, h, r), jbh(XT, h, r),
                                     jbh(AT, h, r))
            Dd = work.tile([128, W], BF, tag="dd")
            nc.scalar.copy(Dd[:, :], psD[:, :])
            q = work.tile([128, W], FP, tag="ffq")
            if qdve:
                nc.vector.tensor_tensor(q[:, :], Dd[:, :], Dd[:, :], ALU.mult)
            else:
                nc.scalar.activation(q[:, :], psD[:, :], AF.Square)
            u = work.tile([128, W], FP, tag="ffu")
            eng("u").tensor_scalar(u[:, :], q[:, :], -1.0, 1.0 + EPS_U,
                                   ALU.mult, ALU.add)
            rr = quake(work, u[:, :], [128, W], "ff", BF)
            zs = work.tile([128, W], BF, tag="zs")
            nc.vector.tensor_tensor(zs[:, :], Dd[:, :], rr[:, :], ALU.mult)
            th = work.tile([128, W], BF, tag="th")
            nc.scalar.activation(th[:, :], zs[:, :], AF.Arctan, scale=-1.0)
            thp = work.tile([128, W], BF, tag="thp")
            nc.vector.tensor_scalar(thp[:, :], th[:, :], HALF_PI, None,
                                    ALU.add)
            f = work.tile([128, W], BF, tag="fv")
            nc.vector.tensor_tensor(f[:, :], thp[:, :], rr[:, :], ALU.mult)
            S = deep.tile([128, W], BF, tag="sg")
            nc.vector.tensor_tensor(S[:, :], w_g[:, :], f[:, :], ALU.mult)
            st["S"] = S


        def emit_factor_smaj(prs, it):
            # per-block matmuls + ACT copies first
            tiles = []
            for st in prs:
                X, XT, AT = st["X"], st["XT"], st["AT"]
                psD = ps_f.tile([128, W], FP, tag="mmf")
                for h in (0, 1):
                    for r in range(R):
                        nc.tensor.matmul(jbh(psD, h, r), jbh(XT, h, r),
                                         jbh(AT, h, r))
                q = work.tile([128, W], FP, tag="ffq")
                nc.scalar.activation(q[:, :], psD[:, :], AF.Square)
                Dd = work.tile([128, W], BF, tag="dd")
                nc.scalar.copy(Dd[:, :], psD[:, :])
                tiles.append((q, Dd))
            # DVE stages interleaved across the pair
            us = []
            for q, Dd in tiles:
                u = work.tile([128, W], FP, tag="ffu")
                nc.vector.tensor_scalar(u[:, :], q[:, :], -1.0, 1.0 + EPS_U,
                                        ALU.mult, ALU.add)
                us.append(u)
            seeds = []
            for u in us:
                seed = work.tile([128, W], FP, tag="ffsd")
                nc.vector.tensor_scalar(seed[:, :].bitcast(I32),
                                        u[:, :].bitcast(I32), 1, -1,
                                        ALU.logical_shift_right,
                                        ALU.bitwise_xor)
                seeds.append(seed)
            rrs = []
            for u, seed in zip(us, seeds):
                rr = work.tile([128, W], BF, tag="ffrr")
                nc.vector._custom_dve(RSQ_F, out=rr[:, :], in0=u[:, :],
                                      in1=seed[:, :], s0=RSQ_C1, s1=RSQ_C2)
                rrs.append(rr)
            zss = []
            for (q, Dd), rr in zip(tiles, rrs):
                zs = work.tile([128, W], BF, tag="zs")
                nc.vector.tensor_tensor(zs[:, :], Dd[:, :], rr[:, :], ALU.mult)
                zss.append(zs)
            ths = []
            for zs in zss:
                th = work.tile([128, W], BF, tag="th")
                nc.scalar.activation(th[:, :], zs[:, :], AF.Arctan, scale=-1.0)
                ths.append(th)
            thps = []
            for th in ths:
                thp = work.tile([128, W], BF, tag="zs")
                nc.vector.tensor_scalar(thp[:, :], th[:, :], HALF_PI, None,
                                        ALU.add)
                thps.append(thp)
            fs = []
            for thp, rr in zip(thps, rrs):
                f = work.tile([128, W], BF, tag="th")
                nc.vector.tensor_tensor(f[:, :], thp[:, :], rr[:, :], ALU.mult)
                fs.append(f)
            for st, f in zip(prs, fs):
                S = deep.tile([128, W], BF, tag="sg")
                nc.vector.tensor_tensor(S[:, :], w_g[:, :], f[:, :], ALU.mult)
                st["S"] = S
        def emit_gmm(st, it):
            X = st["X"]
            psG = ps_f.tile([128, W], FP, tag="mmf")
            if it == 0:
                Xf = st["Xf"]
                for h in (0, 1):
                    for c in (0, 512):
                        nc.tensor.matmul(psG[64 * h:64 * h + 64, c:c + 512],
                                         w_g[64 * h:64 * h + 64, 0:64],
                                         Xf[64 * h:64 * h + 64, c:c + 512])
            else:
                S = st["S"]
                for h in (0, 1):
                    for r in range(R):
                        nc.tensor.matmul(jbh(psG, h, r), jbh(S, h, r),
                                         jbh(X, h, r))
            Gd = deep.tile([128, W], BF, tag="gd")
            nc.scalar.copy(Gd[:, :], psG[:, :])
            # write this block's prod/g2 slices now so psG frees in ACT order
            pg, idx = st["pg"], st["pgidx"]
            off = 2 * W * idx
            if g2dve:
                nc.vector.tensor_tensor(pg[:, off + W:off + 2 * W],
                                        Gd[:, :], Gd[:, :], ALU.mult)
            else:
                nc.scalar.activation(pg[:, off + W:off + 2 * W],
                                     psG[:, :], AF.Square)
            eng("prod").tensor_tensor(pg[:, off:off + W], st["A"][:, :],
                                      Gd[:, :], ALU.mult)
            st["Gd"] = Gd

        def emit_update(sts, it):
            npair = len(sts)
            pg = sts[0]["pg"]
            nred = 2 * R * npair
            red = red_p.tile([128, nred], FP, tag="red")
            # stage 1: pairwise fold at bf16 2x mode (halves reduce volume)
            fold = work.tile([128, W * npair], BF, tag="fold")
            pv = pg[:, :].rearrange("p (s two q) -> p s two q", two=2, q=32)
            nc.vector.tensor_tensor(
                fold[:, :].rearrange("p (s q) -> p s q", q=32),
                pv[:, :, 0, :], pv[:, :, 1, :], ALU.add)
            if fold2:
                fb = work.tile([128, W * npair // 2], BF, tag="fold2")
                fv2 = fold[:, :].rearrange("p (s two q) -> p s two q",
                                           two=2, q=16)
                nc.vector.tensor_tensor(
                    fb[:, :].rearrange("p (s q) -> p s q", q=16),
                    fv2[:, :, 0, :], fv2[:, :, 1, :], ALU.add)
                if fold3:
                    fc = work.tile([128, W * npair // 4], BF, tag="fold3")
                    fv3 = fb[:, :].rearrange("p (s two q) -> p s two q",
                                             two=2, q=8)
                    nc.vector.tensor_tensor(
                        fc[:, :].rearrange("p (s q) -> p s q", q=8),
                        fv3[:, :, 0, :], fv3[:, :, 1, :], ALU.add)
                    fb = fc
                    fview, qq = fb[:, :].rearrange("p (s q) -> p s q", q=8), 8
                else:
                    fview, qq = fb[:, :].rearrange("p (s q) -> p s q", q=16), 16
            else:
                fview, qq = fold[:, :].rearrange("p (s q) -> p s q", q=32), 32
            for c0 in range(0, npair, 2):
                seg = slice(2 * R * c0, 2 * R * (c0 + 2))
                nc.vector.tensor_reduce(
                    red[:, seg].rearrange("p (s j) -> p s j", j=R),
                    fview[:, 2 * R * c0:2 * R * (c0 + 2)], AX.X, ALU.add)
            # red cols: [idx][kind][j]: coefA at kind 0, gnr at kind 1
            rv = red[:, :].rearrange("p (i k j) -> p i k j", k=2, j=R)
            shape = [128, R * npair]
            coefA = red[:, :].rearrange("p (i k j) -> p (i k) j", k=2, j=R)
            # strided views
            cview = rv[:, :, 0, :]          # [128, npair, R]
            gview = rv[:, :, 1, :]
            gn2 = red_p.tile(shape, FP, tag="gn2")
            g3 = gn2[:, :].rearrange("p (i j) -> p i j", j=R)
            nc.vector._custom_dve(GN2_F, out=g3, in0=gview, in1=cview,
                                  s0=1e-30)
            rg = quake(red_p, gn2[:, :], shape, "rg", FP, nr=True)
            gn = red_p.tile(shape, FP, tag="gn")
            nc.vector.tensor_tensor(gn[:, :], gn2[:, :], rg[:, :], ALU.mult)
            cosg = red_p.tile(shape, FP, tag="cosg")
            nc.scalar.activation(cosg[:, :], gn[:, :], AF.Sin,
                                 bias=halfpi[:, 0:1])
            s1t = red_p.tile(shape, FP, tag="s1t")
            nc.scalar.activation(s1t[:, :], gn[:, :], AF.Sin)
            sc = red_p.tile(shape, FP, tag="sc")
            nc.vector.tensor_tensor(sc[:, :], s1t[:, :], rg[:, :], ALU.mult)
            t9 = red_p.tile(shape, FP, tag="t9")
            nc.vector.scalar_tensor_tensor(
                t9[:, :].rearrange("p (i j) -> p i j", j=R), sc[:, :]
                .rearrange("p (i j) -> p i j", j=R), -1.0, cview,
                ALU.mult, ALU.mult)
            alpha = red_p.tile(shape, BF, tag="alpha")
            nc.vector.tensor_tensor(alpha[:, :], cosg[:, :], t9[:, :], ALU.add)
            scb = red_p.tile(shape, BF, tag="scb")
            nc.vector.tensor_copy(scb[:, :], sc[:, :])
            if dup:
                a2 = red_p.tile([128, 2 * R * npair], BF, tag="a2")
                nc.vector.tensor_copy(
                    a2[:, :].rearrange("p (j two) -> p j two", two=2),
                    alpha[:, :].rearrange("p (j o) -> p j o", o=1)
                    .broadcast_to([128, R * npair, 2]))
                s2 = red_p.tile([128, 2 * R * npair], BF, tag="s2")
                nc.vector.tensor_copy(
                    s2[:, :].rearrange("p (j two) -> p j two", two=2),
                    scb[:, :].rearrange("p (j o) -> p j o", o=1)
                    .broadcast_to([128, R * npair, 2]))
            last = it == 2
            for idx, st in enumerate(sts):
                A, Gd = st["A"], st["Gd"]
                t1 = work.tile([128, W], BF, tag="scr1")
                t2 = work.tile([128, W], BF, tag="scr2")
                if dup:
                    def v4(t):
                        return t[:, :].rearrange(
                            "p (j o two) -> p j o two", two=2, o=32)
                    def bc4(small, i0):
                        return small[:, 2 * R * i0:2 * R * i0 + 2 * R] \
                            .rearrange("p (j o two) -> p j o two", o=1, two=2) \
                            .broadcast_to([128, R, 32, 2])
                    nc.vector.tensor_tensor(v4(t1), v4(A), bc4(a2, idx),
                                            ALU.mult)
                    nc.vector.tensor_tensor(v4(t2), v4(Gd), bc4(s2, idx),
                                            ALU.mult)
                else:
                    eng("t1").tensor_tensor(b3(t1), b3(A),
                                            bcR(alpha[:, R * idx:R * idx + R]),
                                            ALU.mult)
                    eng("t2").tensor_tensor(b3(t2), b3(Gd),
                                            bcR(scb[:, R * idx:R * idx + R]),
                                            ALU.mult)
                An = ab_p.tile([128, W], BF, tag="agf" if last else "ag")
                nc.vector.tensor_tensor(An[:, :], t1[:, :], t2[:, :], ALU.add)
                st["A"] = An
                if not last:
                    tp = ps_t.tile([128, W], BF, tag="tp")
                    for h in (0, 1):
                        for r in range(R):
                            nc.tensor.transpose(jbh(tp, h, r), jbh(An, h, r),
                                                ident[64 * h:64 * h + 64, :])
                    AT = ab_p.tile([128, W], BF, tag="at")
                    (nc.vector.tensor_copy if cpdve else nc.scalar.copy)(
                        AT[:, :], tp[:, :])
                    st["AT"] = AT
                else:
                    for h in (0, 1):
                        nc.sync.dma_start(
                            out_d[st["b"], h],
                            b3(An[64 * h:64 * h + 64, :]))

        all_sts = [{"b": b} for b in range(NBLK)]
        for st in all_sts:
            emit_load(st)
            emit_d0(st)
        for b0 in range(0, NBLK, INTERLEAVE):
            sts = all_sts[b0:b0 + INTERLEAVE]
            for it in range(3):
                pg = work.tile([128, 2 * W * len(sts)], BF, tag="pg")
                for idx, st in enumerate(sts):
                    st["pg"], st["pgidx"] = pg, idx
                if smaj and it > 0:
                    for i0 in range(0, len(sts), 2):
                        emit_factor_smaj(sts[i0:i0 + 2], it)
                    for st in sts:
                        emit_gmm(st, it)
                elif stagger:
                    n = len(sts)
                    for k in range(n + 1):
                        if k < n:
                            emit_factor(sts[k], it)
                        if k > 0:
                            emit_gmm(sts[k - 1], it)
                else:
                    for st in sts:
                        emit_factor(st, it)
                    for st in sts:
                        emit_gmm(st, it)
                emit_update(sts, it)
    nc.compile()
    return nc


def _get_program(**kw):
    key = tuple(sorted((k, tuple(v) if isinstance(v, (list, tuple, set, frozenset))
                        else v) for k, v in kw.items()))
    if key not in _COMPILED:
        _COMPILED[key] = build_program(**kw)
    return _COMPILED[key]


def kernel(x, w_raw, _trace=False, **bkw):
    import ml_dtypes
    from concourse.bass_utils import run_bass_kernel_spmd
    if _trace:
        _trace = _ensure_trace_hook()

    bf16 = ml_dtypes.bfloat16
    x = np.asarray(x, f32)
    w_raw = np.asarray(w_raw, f32)
    B, L, C_in, d = x.shape
    N = B * L
    w = np.exp((w_raw - f32(np.log(C_in))).astype(f32)).astype(f32)
    w = (w / w.sum(axis=0, keepdims=True)).astype(f32)

    xr = x.reshape(N, C_in, d)
    # per core: [NBLK, 2, R, i, d] -> transpose to [NBLK, 2, i, R, d]
    xcore = xr.reshape(N_CORES, NBLK, 2, R, C_in, d)
    xp = np.ascontiguousarray(xcore.transpose(0, 1, 2, 4, 3, 5)).astype(bf16)
    x0p = np.ascontiguousarray(xcore[:, :, :, :, 0, :]).astype(bf16)
    w_rep = np.ascontiguousarray(
        np.broadcast_to(w.T.reshape(1, 64, 1, 64), (2, 64, R, 64))
        .transpose(0, 3, 2, 1).reshape(128, W)).astype(bf16)
    # w_rep[p, (j, o)]: lower/upper halves identical, = w[i=p%64, o]
    w_rep = np.ascontiguousarray(
        np.tile(np.repeat(w[None, :, :], 1, axis=0), (2, 1, 1))  # (2,64,64)
        .reshape(2, 64, 1, 64).repeat(R, axis=2).reshape(2 * 64, R * 64)
        ).astype(bf16)
    ident2 = np.tile(np.eye(64, dtype=bf16), (2, 1))

    nc = _get_program(**bkw)
    in_maps = []
    for k in range(N_CORES):
        in_maps.append({
            "xp": xp[k],
            "x0p": x0p[k],
            "w_rep": w_rep,
            "ident2": ident2,
        })
    res = run_bass_kernel_spmd(nc, in_maps, core_ids=list(range(N_CORES)),
                               trace=_trace)
    # out_p: [NBLK, 2, o, j, d] per core -> rows
    outs = []
    for k in range(N_CORES):
        op = res.results[k]["out_p"]          # (NBLK, 2, 64, R, 64)
        outs.append(np.ascontiguousarray(op.transpose(0, 1, 3, 2, 4))
                    .reshape(ROWS_PER_CORE, C_OUT, d))
    out = np.concatenate(outs, axis=0)
    if _trace:
        kernel.last_exec_time_ns = res.exec_time_ns
        kernel.last_results = res
    return out.reshape(B, L, C_OUT, d).astype(f32)


# revision 9
# speedup vs baseline: 1.0345x; 1.0345x over previous
"""Trainium2 Bass kernel v3 for nn_MfdFC. See kernel_v2 docstring for math.

v3 over v2:
- host pre-transposes x into per-(block,half) [i, j, d] layout and w into the
  replicated [128, 1024] SBUF image -> all input DMAs are contiguous; the
  output is written in SBUF-natural [o, j, d] order and re-transposed on host.
- the per-block [128,16] "smalls" pipelines (iter-0 f-chain, update-phase
  cos/sin/rsqrt chain) run once per interleave-PAIR on [128,32] tiles, and
  the coefA/|G|^2 reductions of a pair are fused into ONE 4096-wide reduce.
- optional GPSIMD offload for selected elementwise passes (t2, prod).
"""
import math
import numpy as np

f32 = np.float32

C_IN = 64
C_OUT = 64
D_DIM = 64
ROWS_PER_CORE = 128
N_CORES = 8
R = 16
NBLK = 4
W = 64 * R
RSQ_C1 = 1.7584694439735017e-30
RSQ_C2 = -2.755803843779718e-20
HALF_PI = float(f32(math.pi / 2.0))
EPS_U = float(f32(2.0 ** -22))

_COMPILED = {}

def _register_custom_ops():
    import concourse.dve_ops as dve_ops
    from concourse.dve_ops import DveOp
    from concourse.dve_spec import (
        Spec, Src0, Src1, C0, C1, lower, maxx, _has_src1 as has_src1,
    )
    from concourse.dve_uop import DveOpSpec
    from concourse.dve_table_gen import dve_ver_for

    if "ANT_RSQ_F" in dve_ops._SUB_OPCODE_FOR_NAME:
        return {n: op for n, op in ((o.name, o) for o in dve_ops.OPS)
                if n.startswith("ANT_")}

    def _ref_rsq_f(in0, in1, s0, s1, imm2):
        u = np.asarray(in0, f32)
        nt = np.asarray(in1, f32)
        m1 = (nt * f32(s0)).astype(f32)
        m2 = (m1 * nt).astype(f32)
        m3 = (m2 * f32(s0)).astype(f32)
        t = (m3 * u).astype(f32)
        return ((t + f32(s1)) * nt).astype(f32)

    _m1 = Src1 * C0
    _m3 = (_m1 * Src1) * C0
    RSQ_F = DveOp("ANT_RSQ_F",
                  Spec(body=((_m3 * Src0) + C1) * Src1, reference=_ref_rsq_f),
                  subdim=False, uops_sha={})

    def _ref_rsq_nr(in0, in1, s0, s1, imm2):
        u = np.asarray(in0, f32); y = np.asarray(in1, f32)
        a = (u * y).astype(f32)
        b = (a * y).astype(f32)
        return ((f32(s0) - (b * f32(s1)).astype(f32)) * y).astype(f32)

    RSQ_NR = DveOp("ANT_RSQ_NR",
                   Spec(body=(C0 - ((Src0 * Src1) * Src1) * C1) * Src1,
                        reference=_ref_rsq_nr),
                   subdim=False, uops_sha={})

    def _ref_gn2(in0, in1, s0, s1, imm2):
        raw = np.asarray(in0, f32); c = np.asarray(in1, f32)
        return np.maximum((raw - (c * c).astype(f32)).astype(f32), f32(s0))

    GN2_F = DveOp("ANT_GN2_F",
                  Spec(body=maxx(Src0 - Src1 * Src1, C0), reference=_ref_gn2),
                  subdim=False, uops_sha={})

    ops = [RSQ_F, RSQ_NR, GN2_F]
    base = dve_ops._CUSTOM_DVE_ROW_BASE + len(dve_ops.OPS)
    for i, op in enumerate(ops):
        dve_ops._SUB_OPCODE_FOR_NAME[op.name] = base + i
    for trn in ("TRN2",):
        ver = dve_ver_for(trn)
        for op in ops:
            uops = lower(op.spec, ver=ver)
            s = DveOpSpec(name=op.name, opcode=dve_ops.get_dve_sub_opcode(op.name),
                          uops=uops, rd1_en=has_src1(op.spec))
            op.uops_sha[ver] = s.sha(ver)
    dve_ops.OPS.extend(ops)
    dve_ops.CUSTOM_DVE_SPECS.update({op.name: op.spec for op in ops})
    return {op.name: op for op in ops}



def _ensure_trace_hook():
    try:
        from antenv.axon_hooks import get_axon_ntff_profile_hook
        return get_axon_ntff_profile_hook() is not None
    except ImportError:
        pass
    try:
        import sys, types
        import antenv
        from trn_agent_boot.trn_boot import _ntff_profile_via_ctypes
        mod = types.ModuleType("antenv.axon_hooks")
        _h = {}
        mod.set_axon_ntff_profile_hook = lambda h: _h.__setitem__("h", h)
        mod.get_axon_ntff_profile_hook = lambda: _h.get("h")
        sys.modules["antenv.axon_hooks"] = mod
        antenv.axon_hooks = mod
        mod.set_axon_ntff_profile_hook(
            _ntff_profile_via_ctypes("/opt/axon/libaxon_pjrt.so"))
        return True
    except Exception:
        return False





def build_program(INTERLEAVE=4, gps=(), redsplit=False, wbufs=2,
                  stagger=False, psf=2, pst=2, pss=2, dbufs=None, fold2=True, fold3=False, dup=True, qdve=False, g2dve=False, cpdve=False, smaj=False):
    from contextlib import ExitStack
    import concourse.bacc as bacc
    import concourse.mybir as mybir
    import concourse.tile as tile

    gps = frozenset(gps)
    FP = mybir.dt.float32
    BF = mybir.dt.bfloat16
    I32 = mybir.dt.int32
    AF = mybir.ActivationFunctionType
    ALU = mybir.AluOpType
    AX = mybir.AxisListType

    OPS = _register_custom_ops()
    RSQ_F, RSQ_NR, GN2_F = OPS["ANT_RSQ_F"], OPS["ANT_RSQ_NR"], OPS["ANT_GN2_F"]

    nc = bacc.Bacc()
    # x pre-transposed on host: [block, half, i, j, d]
    x_d = nc.dram_tensor("xp", (NBLK, 2, C_IN, R, D_DIM), BF,
                         kind="ExternalInput")
    # x0 rows: [block, half, j, d]
    x0_d = nc.dram_tensor("x0p", (NBLK, 2, R, D_DIM), BF, kind="ExternalInput")
    w_d = nc.dram_tensor("w_rep", (128, W), BF, kind="ExternalInput")
    id_d = nc.dram_tensor("ident2", (128, 64), BF, kind="ExternalInput")
    # output in SBUF-natural order: [block, half, o, j, d]
    out_d = nc.dram_tensor("out_p", (NBLK, 2, C_OUT, R, D_DIM), BF,
                           kind="ExternalOutput")

    ctx = ExitStack()
    with ctx:
        tc = ctx.enter_context(tile.TileContext(nc))
        const = ctx.enter_context(tc.tile_pool(name="const", bufs=1))
        xg_p = ctx.enter_context(tc.tile_pool(name="xg", bufs=NBLK))
        work = ctx.enter_context(tc.tile_pool(name="work", bufs=wbufs))
        deep = ctx.enter_context(tc.tile_pool(name="deep", bufs=dbufs or max(2, INTERLEAVE)))
        ab_p = ctx.enter_context(tc.tile_pool(name="ab", bufs=NBLK))
        red_p = ctx.enter_context(tc.tile_pool(name="red", bufs=max(2, INTERLEAVE)))
        ps_f = ctx.enter_context(tc.tile_pool(name="psf", bufs=psf, space="PSUM"))
        ps_t = ctx.enter_context(tc.tile_pool(name="pst", bufs=pst, space="PSUM"))
        ps_s = ctx.enter_context(tc.tile_pool(name="pss", bufs=pss, space="PSUM"))

        def eng(name):
            return nc.gpsimd if name in gps else nc.vector

        # ---- constants (all contiguous DMAs)
        w_g = const.tile([128, W], BF, tag="wg")
        nc.sync.dma_start(w_g[:, :], w_d[:, :])
        ident = const.tile([128, 64], BF, tag="ident")
        nc.sync.dma_start(ident[:, :], id_d[:, :])
        halfpi = const.tile([128, 1], FP, tag="halfpi")
        nc.vector.memset(halfpi[:, :], HALF_PI)

        def jbh(t, h, j):
            return t[64 * h:64 * h + 64, 64 * j:64 * j + 64]

        def b3(t):
            return t[:, :].rearrange("p (j d) -> p j d", d=64)

        def bcR(small_ap):      # [128, R] ap -> broadcast (p, j, 64)
            return small_ap.rearrange("p (j o) -> p j o", o=1) \
                .broadcast_to([128, R, 64])

        def emit_load(st):
            b = st["b"]
            X = xg_p.tile([128, W], BF, tag="xg")
            for h in (0, 1):
                nc.sync.dma_start(b3(X[64 * h:64 * h + 64, :]), x_d[b, h])
            A0 = ab_p.tile([128, W], BF, tag="a0")
            for h in (0, 1):
                nc.sync.dma_start(
                    A0[64 * h:64 * h + 64, :].rearrange("p (j d) -> p j d", d=64),
                    x0_d[b:b + 1, h].rearrange("b j d -> b j d")
                    .broadcast_to([64, R, 64]))
            tp = ps_t.tile([128, W], BF, tag="tp")
            for h in (0, 1):
                for r in range(R):
                    nc.tensor.transpose(jbh(tp, h, r), jbh(X, h, r),
                                        ident[64 * h:64 * h + 64, :])
            XT = xg_p.tile([128, W], BF, tag="xt")
            (nc.vector.tensor_copy if cpdve else nc.scalar.copy)(
                XT[:, :], tp[:, :])
            st["X"], st["XT"], st["A"] = X, XT, A0

        def quake(pool, src_ap, shape, tagp, out_dt, nr=False):
            seed = pool.tile(shape, FP, tag=tagp + "sd")
            nc.vector.tensor_scalar(seed[:, :].bitcast(I32),
                                    src_ap.bitcast(I32), 1, -1,
                                    ALU.logical_shift_right, ALU.bitwise_xor)
            rr = pool.tile(shape, out_dt, tag=tagp + "rr")
            nc.vector._custom_dve(RSQ_F, out=rr[:, :], in0=src_ap,
                                  in1=seed[:, :], s0=RSQ_C1, s1=RSQ_C2)
            if not nr:
                return rr
            rr2 = pool.tile(shape, out_dt, tag=tagp + "r2")
            nc.vector._custom_dve(RSQ_NR, out=rr2[:, :], in0=src_ap,
                                  in1=rr[:, :], s0=1.5, s1=0.5)
            return rr2

        # ---------- iteration 0: per-block D0 + small f-chain
        def emit_d0(st):
            XT = st["XT"]
            psD0 = ps_s.tile([128, R], FP, tag="d0")
            for h in (0, 1):
                for r in range(R):
                    nc.tensor.matmul(
                        psD0[64 * h:64 * h + 64, r:r + 1],
                        jbh(XT, h, r),
                        XT[64 * h:64 * h + 64, 64 * r:64 * r + 1])
            shape = [128, R]
            q0 = red_p.tile(shape, FP, tag="f0q")
            nc.scalar.activation(q0[:, :], psD0[:, :], AF.Square)
            u0 = red_p.tile(shape, FP, tag="f0u")
            nc.vector.tensor_scalar(u0[:, :], q0[:, :], -1.0, 1.0 + EPS_U,
                                    ALU.mult, ALU.add)
            rr0 = quake(red_p, u0[:, :], shape, "f0", FP)
            zs0 = red_p.tile(shape, FP, tag="f0z")
            nc.vector.tensor_tensor(zs0[:, :], psD0[:, :], rr0[:, :], ALU.mult)
            th0 = red_p.tile(shape, FP, tag="f0t")
            nc.scalar.activation(th0[:, :], zs0[:, :], AF.Arctan, scale=-1.0)
            f0 = red_p.tile(shape, BF, tag="f0v")
            nc.vector.scalar_tensor_tensor(f0[:, :], th0[:, :], HALF_PI,
                                           rr0[:, :], ALU.add, ALU.mult)
            st["f0"] = f0[:, :]

        def emit_factor(st, it):
            X, XT = st["X"], st["XT"]
            if it == 0:
                Xf = deep.tile([128, W], BF, tag="xf")
                nc.vector.tensor_tensor(b3(Xf), b3(X), bcR(st["f0"]), ALU.mult)
                st["Xf"] = Xf
                return
            AT = st["AT"]
            psD = ps_f.tile([128, W], FP, tag="mmf")
            for h in (0, 1):
                for r in range(R):
                    nc.tensor.matmul(jbh(psD, h, r), jbh(XT, h, r),
                                     jbh(AT, h, r))
            q = work.tile([128, W], FP, tag="ffq")
            if not qdve:
                nc.scalar.activation(q[:, :], psD[:, :], AF.Square)
            Dd = work.tile([128, W], BF, tag="dd")
            nc.scalar.copy(Dd[:, :], psD[:, :])
            if qdve:
                nc.vector.tensor_tensor(q[:, :], Dd[:, :], Dd[:, :], ALU.mult)
            u = work.tile([128, W], FP, tag="ffu")
            eng("u").tensor_scalar(u[:, :], q[:, :], -1.0, 1.0 + EPS_U,
                                   ALU.mult, ALU.add)
            rr = quake(work, u[:, :], [128, W], "ff", BF)
            zs = work.tile([128, W], BF, tag="zs")
            nc.vector.tensor_tensor(zs[:, :], Dd[:, :], rr[:, :], ALU.mult)
            th = work.tile([128, W], BF, tag="th")
            nc.scalar.activation(th[:, :], zs[:, :], AF.Arctan, scale=-1.0)
            thp = work.tile([128, W], BF, tag="thp")
            nc.vector.tensor_scalar(thp[:, :], th[:, :], HALF_PI, None,
                                    ALU.add)
            f = work.tile([128, W], BF, tag="fv")
            nc.vector.tensor_tensor(f[:, :], thp[:, :], rr[:, :], ALU.mult)
            S = deep.tile([128, W], BF, tag="sg")
            nc.vector.tensor_tensor(S[:, :], w_g[:, :], f[:, :], ALU.mult)
            st["S"] = S


        def emit_factor_smaj(prs, it):
            # per-block matmuls + ACT copies first
            tiles = []
            for st in prs:
                X, XT, AT = st["X"], st["XT"], st["AT"]
                psD = ps_f.tile([128, W], FP, tag="mmf")
                for h in (0, 1):
                    for r in range(R):
                        nc.tensor.matmul(jbh(psD, h, r), jbh(XT, h, r),
                                         jbh(AT, h, r))
                q = work.tile([128, W], FP, tag="ffq")
                nc.scalar.activation(q[:, :], psD[:, :], AF.Square)
                Dd = work.tile([128, W], BF, tag="dd")
                nc.scalar.copy(Dd[:, :], psD[:, :])
                tiles.append((q, Dd))
            # DVE stages interleaved across the pair
            us = []
            for q, Dd in tiles:
                u = work.tile([128, W], FP, tag="ffu")
                nc.vector.tensor_scalar(u[:, :], q[:, :], -1.0, 1.0 + EPS_U,
                                        ALU.mult, ALU.add)
                us.append(u)
            seeds = []
            for u in us:
                seed = work.tile([128, W], FP, tag="ffsd")
                nc.vector.tensor_scalar(seed[:, :].bitcast(I32),
                                        u[:, :].bitcast(I32), 1, -1,
                                        ALU.logical_shift_right,
                                        ALU.bitwise_xor)
                seeds.append(seed)
            rrs = []
            for u, seed in zip(us, seeds):
                rr = work.tile([128, W], BF, tag="ffrr")
                nc.vector._custom_dve(RSQ_F, out=rr[:, :], in0=u[:, :],
                                      in1=seed[:, :], s0=RSQ_C1, s1=RSQ_C2)
                rrs.append(rr)
            zss = []
            for (q, Dd), rr in zip(tiles, rrs):
                zs = work.tile([128, W], BF, tag="zs")
                nc.vector.tensor_tensor(zs[:, :], Dd[:, :], rr[:, :], ALU.mult)
                zss.append(zs)
            ths = []
            for zs in zss:
                th = work.tile([128, W], BF, tag="th")
                nc.scalar.activation(th[:, :], zs[:, :], AF.Arctan, scale=-1.0)
                ths.append(th)
            thps = []
            for th in ths:
                thp = work.tile([128, W], BF, tag="zs")
                nc.vector.tensor_scalar(thp[:, :], th[:, :], HALF_PI, None,
                                        ALU.add)
                thps.append(thp)
            fs = []
            for thp, rr in zip(thps, rrs):
                f = work.tile([128, W], BF, tag="th")
                nc.vector.tensor_tensor(f[:, :], thp[:, :], rr[:, :], ALU.mult)
                fs.append(f)
            for st, f in zip(prs, fs):
                S = deep.tile([128, W], BF, tag="sg")
                nc.vector.tensor_tensor(S[:, :], w_g[:, :], f[:, :], ALU.mult)
                st["S"] = S
        def emit_gmm(st, it):
            X = st["X"]
            psG = ps_f.tile([128, W], FP, tag="mmf")
            if it == 0:
                Xf = st["Xf"]
                for h in (0, 1):
                    for c in (0, 512):
                        nc.tensor.matmul(psG[64 * h:64 * h + 64, c:c + 512],
                                         w_g[64 * h:64 * h + 64, 0:64],
                                         Xf[64 * h:64 * h + 64, c:c + 512])
            else:
                S = st["S"]
                for h in (0, 1):
                    for r in range(R):
                        nc.tensor.matmul(jbh(psG, h, r), jbh(S, h, r),
                                         jbh(X, h, r))
            Gd = deep.tile([128, W], BF, tag="gd")
            nc.scalar.copy(Gd[:, :], psG[:, :])
            # write this block's prod/g2 slices now so psG frees in ACT order
            pg, idx = st["pg"], st["pgidx"]
            off = 2 * W * idx
            if g2dve:
                nc.vector.tensor_tensor(pg[:, off + W:off + 2 * W],
                                        Gd[:, :], Gd[:, :], ALU.mult)
            else:
                nc.scalar.activation(pg[:, off + W:off + 2 * W],
                                     psG[:, :], AF.Square)
            eng("prod").tensor_tensor(pg[:, off:off + W], st["A"][:, :],
                                      Gd[:, :], ALU.mult)
            st["Gd"] = Gd

        def emit_update(sts, it):
            npair = len(sts)
            pg = sts[0]["pg"]
            nred = 2 * R * npair
            red = red_p.tile([128, nred], FP, tag="red")
            # stage 1: pairwise fold at bf16 2x mode (halves reduce volume)
            fold = work.tile([128, W * npair], BF, tag="fold")
            pv = pg[:, :].rearrange("p (s two q) -> p s two q", two=2, q=32)
            nc.vector.tensor_tensor(
                fold[:, :].rearrange("p (s q) -> p s q", q=32),
                pv[:, :, 0, :], pv[:, :, 1, :], ALU.add)
            if fold2:
                fb = work.tile([128, W * npair // 2], BF, tag="fold2")
                fv2 = fold[:, :].rearrange("p (s two q) -> p s two q",
                                           two=2, q=16)
                nc.vector.tensor_tensor(
                    fb[:, :].rearrange("p (s q) -> p s q", q=16),
                    fv2[:, :, 0, :], fv2[:, :, 1, :], ALU.add)
                if fold3:
                    fc = work.tile([128, W * npair // 4], BF, tag="fold3")
                    fv3 = fb[:, :].rearrange("p (s two q) -> p s two q",
                                             two=2, q=8)
                    nc.vector.tensor_tensor(
                        fc[:, :].rearrange("p (s q) -> p s q", q=8),
                        fv3[:, :, 0, :], fv3[:, :, 1, :], ALU.add)
                    fb = fc
                    fview, qq = fb[:, :].rearrange("p (s q) -> p s q", q=8), 8
                else:
                    fview, qq = fb[:, :].rearrange("p (s q) -> p s q", q=16), 16
            else:
                fview, qq = fold[:, :].rearrange("p (s q) -> p s q", q=32), 32
            for c0 in range(0, npair, 2):
                seg = slice(2 * R * c0, 2 * R * (c0 + 2))
                nc.vector.tensor_reduce(
                    red[:, seg].rearrange("p (s j) -> p s j", j=R),
                    fview[:, 2 * R * c0:2 * R * (c0 + 2)], AX.X, ALU.add)
            # red cols: [idx][kind][j]: coefA at kind 0, gnr at kind 1
            rv = red[:, :].rearrange("p (i k j) -> p i k j", k=2, j=R)
            shape = [128, R * npair]
            coefA = red[:, :].rearrange("p (i k j) -> p (i k) j", k=2, j=R)
            # strided views
            cview = rv[:, :, 0, :]          # [128, npair, R]
            gview = rv[:, :, 1, :]
            gn2 = red_p.tile(shape, FP, tag="gn2")
            g3 = gn2[:, :].rearrange("p (i j) -> p i j", j=R)
            nc.vector._custom_dve(GN2_F, out=g3, in0=gview, in1=cview,
                                  s0=1e-30)
            rg = quake(red_p, gn2[:, :], shape, "rg", FP, nr=True)
            gn = red_p.tile(shape, FP, tag="gn")
            nc.vector.tensor_tensor(gn[:, :], gn2[:, :], rg[:, :], ALU.mult)
            cosg = red_p.tile(shape, FP, tag="cosg")
            nc.scalar.activation(cosg[:, :], gn[:, :], AF.Sin,
                                 bias=halfpi[:, 0:1])
            s1t = red_p.tile(shape, FP, tag="s1t")
            nc.scalar.activation(s1t[:, :], gn[:, :], AF.Sin)
            sc = red_p.tile(shape, FP, tag="sc")
            nc.vector.tensor_tensor(sc[:, :], s1t[:, :], rg[:, :], ALU.mult)
            t9 = red_p.tile(shape, FP, tag="t9")
            nc.vector.scalar_tensor_tensor(
                t9[:, :].rearrange("p (i j) -> p i j", j=R), sc[:, :]
                .rearrange("p (i j) -> p i j", j=R), -1.0, cview,
                ALU.mult, ALU.mult)
            alpha = red_p.tile(shape, BF, tag="alpha")
            nc.vector.tensor_tensor(alpha[:, :], cosg[:, :], t9[:, :], ALU.add)
            scb = red_p.tile(shape, BF, tag="scb")
            nc.vector.tensor_copy(scb[:, :], sc[:, :])
            if dup:
                a2 = red_p.tile([128, 2 * R * npair], BF, tag="a2")
                nc.vector.tensor_copy(
                    a2[:, :].rearrange("p (j two) -> p j two", two=2),
                    alpha[:, :].rearrange("p (j o) -> p j o", o=1)
                    .broadcast_to([128, R * npair, 2]))
                s2 = red_p.tile([128, 2 * R * npair], BF, tag="s2")
                nc.vector.tensor_copy(
                    s2[:, :].rearrange("p (j two) -> p j two", two=2),
                    scb[:, :].rearrange("p (j o) -> p j o", o=1)
                    .broadcast_to([128, R * npair, 2]))
            last = it == 2
            for idx, st in enumerate(sts):
                A, Gd = st["A"], st["Gd"]
                t1 = work.tile([128, W], BF, tag="scr1")
                t2 = work.tile([128, W], BF, tag="scr2")
                if dup:
                    def v4(t):
                        return t[:, :].rearrange(
                            "p (j o two) -> p j o two", two=2, o=32)
                    def bc4(small, i0):
                        return small[:, 2 * R * i0:2 * R * i0 + 2 * R] \
                            .rearrange("p (j o two) -> p j o two", o=1, two=2) \
                            .broadcast_to([128, R, 32, 2])
                    nc.vector.tensor_tensor(v4(t1), v4(A), bc4(a2, idx),
                                            ALU.mult)
                    nc.vector.tensor_tensor(v4(t2), v4(Gd), bc4(s2, idx),
                                            ALU.mult)
                else:
                    eng("t1").tensor_tensor(b3(t1), b3(A),
                                            bcR(alpha[:, R * idx:R * idx + R]),
                                            ALU.mult)
                    eng("t2").tensor_tensor(b3(t2), b3(Gd),
                                            bcR(scb[:, R * idx:R * idx + R]),
                                            ALU.mult)
                An = ab_p.tile([128, W], BF, tag="agf" if last else "ag")
                nc.vector.tensor_tensor(An[:, :], t1[:, :], t2[:, :], ALU.add)
                st["A"] = An
                if not last:
                    tp = ps_t.tile([128, W], BF, tag="tp")
                    for h in (0, 1):
                        for r in range(R):
                            nc.tensor.transpose(jbh(tp, h, r), jbh(An, h, r),
                                                ident[64 * h:64 * h + 64, :])
                    AT = ab_p.tile([128, W], BF, tag="at")
                    (nc.vector.tensor_copy if cpdve else nc.scalar.copy)(
                        AT[:, :], tp[:, :])
                    st["AT"] = AT
                else:
                    for h in (0, 1):
                        nc.sync.dma_start(
                            out_d[st["b"], h],
                            b3(An[64 * h:64 * h + 64, :]))

        all_sts = [{"b": b} for b in range(NBLK)]
        for st in all_sts:
            emit_load(st)
            emit_d0(st)
        for b0 in range(0, NBLK, INTERLEAVE):
            sts = all_sts[b0:b0 + INTERLEAVE]
            for it in range(3):
                pg = work.tile([128, 2 * W * len(sts)], BF, tag="pg")
                for idx, st in enumerate(sts):
                    st["pg"], st["pgidx"] = pg, idx
                if smaj and it > 0:
                    for i0 in range(0, len(sts), 2):
                        emit_factor_smaj(sts[i0:i0 + 2], it)
                    for st in sts:
                        emit_gmm(st, it)
                elif stagger:
                    n = len(sts)
                    for k in range(n + 1):
                        if k < n:
                            emit_factor(sts[k], it)
                        if k > 0:
                            emit_gmm(sts[k - 1], it)
                else:
                    for st in sts:
                        emit_factor(st, it)
                    for st in sts:
                        emit_gmm(st, it)
                emit_update(sts, it)
    nc.compile()
    return nc


def _get_program(**kw):
    key = tuple(sorted((k, tuple(v) if isinstance(v, (list, tuple, set, frozenset))
                        else v) for k, v in kw.items()))
    if key not in _COMPILED:
        _COMPILED[key] = build_program(**kw)
    return _COMPILED[key]


def kernel(x, w_raw, _trace=False, **bkw):
    import ml_dtypes
    from concourse.bass_utils import run_bass_kernel_spmd
    if _trace:
        _trace = _ensure_trace_hook()

    bf16 = ml_dtypes.bfloat16
    x = np.asarray(x, f32)
    w_raw = np.asarray(w_raw, f32)
    B, L, C_in, d = x.shape
    N = B * L
    w = np.exp((w_raw - f32(np.log(C_in))).astype(f32)).astype(f32)
    w = (w / w.sum(axis=0, keepdims=True)).astype(f32)

    xr = x.reshape(N, C_in, d)
    # per core: [NBLK, 2, R, i, d] -> transpose to [NBLK, 2, i, R, d]
    xcore = xr.reshape(N_CORES, NBLK, 2, R, C_in, d)
    xp = np.ascontiguousarray(xcore.transpose(0, 1, 2, 4, 3, 5)).astype(bf16)
    x0p = np.ascontiguousarray(xcore[:, :, :, :, 0, :]).astype(bf16)
    w_rep = np.ascontiguousarray(
        np.broadcast_to(w.T.reshape(1, 64, 1, 64), (2, 64, R, 64))
        .transpose(0, 3, 2, 1).reshape(128, W)).astype(bf16)
    # w_rep[p, (j, o)]: lower/upper halves identical, = w[i=p%64, o]
    w_rep = np.ascontiguousarray(
        np.tile(np.repeat(w[None, :, :], 1, axis=0), (2, 1, 1))  # (2,64,64)
        .reshape(2, 64, 1, 64).repeat(R, axis=2).reshape(2 * 64, R * 64)
        ).astype(bf16)
    ident2 = np.tile(np.eye(64, dtype=bf16), (2, 1))

    nc = _get_program(**bkw)
    in_maps = []
    for k in range(N_CORES):
        in_maps.append({
            "xp": xp[k],
            "x0p": x0p[k],
            "w_rep": w_rep,
            "ident2": ident2,
        })
    res = run_bass_kernel_spmd(nc, in_maps, core_ids=list(range(N_CORES)),
                               trace=_trace)
    # out_p: [NBLK, 2, o, j, d] per core -> rows
    outs = []
    for k in range(N_CORES):
        op = res.results[k]["out_p"]          # (NBLK, 2, 64, R, 64)
        outs.append(np.ascontiguousarray(op.transpose(0, 1, 3, 2, 4))
                    .reshape(ROWS_PER_CORE, C_OUT, d))
    out = np.concatenate(outs, axis=0)
    if _trace:
        kernel.last_exec_time_ns = res.exec_time_ns
        kernel.last_results = res
    return out.reshape(B, L, C_OUT, d).astype(f32)


# revision 10
# speedup vs baseline: 1.0345x; 1.0001x over previous
"""Trainium2 Bass kernel for nn_MfdFC (spherical weighted-Frechet-mean).

Math per row n (N=1024, 128 rows/core): w = col-softmax(w_raw); a(o) <- x0;
3 iterations of  D = <a_o, x_i>;  f = (pi/2 + arctan(-D*rr))*rr with
rr = rsqrt(1+eps-D^2) (quake rsqrt on DVE, arctan on ACT);  S = w^T。f;
G = S @ X;  c = sum_d A。G;  gn = sqrt(sum G^2 - c^2);
a <- (cos gn - sinc(gn) c) a + sinc(gn) G.

Design: all matmuls bf16 (4x PE rate); elementwise bf16 in DVE 2x packed
mode. 128 rows/core as 4 blocks of 32, halves stacked at PSUM partitions
0-63/64-127 (PE writes upper partitions directly); all four blocks
pipelined (INTERLEAVE=4) with deep-rotation pools so tile reuse cannot
cycle against the in-order engine queues. Host pre-transposes x and
pre-replicates w (contiguous DMAs); output returns bf16 and is cast on
host. Iteration 0 specialized (a==x0). The coefA/|G|^2 reductions run as
a tree: two bf16 pairwise folds at 2x rate, then a 1x tensor_reduce on
16x less data. The alpha/sinc scalars are stored duplicated in adjacent
pairs so their d-broadcast APs have stride-1 2-element inner runs --
which qualifies the t1/t2 multiplies for the DVE's packed 2x mode.
q (Square) is emitted before the D-copy so ACT serves the longer DVE
rsqrt chain first.
"""
import math
import numpy as np

f32 = np.float32

C_IN = 64
C_OUT = 64
D_DIM = 64
ROWS_PER_CORE = 128
N_CORES = 8
R = 16
NBLK = 4
W = 64 * R
RSQ_C1 = 1.7584694439735017e-30
RSQ_C2 = -2.755803843779718e-20
HALF_PI = float(f32(math.pi / 2.0))
EPS_U = float(f32(2.0 ** -22))

_COMPILED = {}

def _register_custom_ops():
    import concourse.dve_ops as dve_ops
    from concourse.dve_ops import DveOp
    from concourse.dve_spec import (
        Spec, Src0, Src1, C0, C1, lower, maxx, _has_src1 as has_src1,
    )
    from concourse.dve_uop import DveOpSpec
    from concourse.dve_table_gen import dve_ver_for

    if "ANT_RSQ_F" in dve_ops._SUB_OPCODE_FOR_NAME:
        return {n: op for n, op in ((o.name, o) for o in dve_ops.OPS)
                if n.startswith("ANT_")}

    def _ref_rsq_f(in0, in1, s0, s1, imm2):
        u = np.asarray(in0, f32)
        nt = np.asarray(in1, f32)
        m1 = (nt * f32(s0)).astype(f32)
        m2 = (m1 * nt).astype(f32)
        m3 = (m2 * f32(s0)).astype(f32)
        t = (m3 * u).astype(f32)
        return ((t + f32(s1)) * nt).astype(f32)

    _m1 = Src1 * C0
    _m3 = (_m1 * Src1) * C0
    RSQ_F = DveOp("ANT_RSQ_F",
                  Spec(body=((_m3 * Src0) + C1) * Src1, reference=_ref_rsq_f),
                  subdim=False, uops_sha={})

    def _ref_rsq_nr(in0, in1, s0, s1, imm2):
        u = np.asarray(in0, f32); y = np.asarray(in1, f32)
        a = (u * y).astype(f32)
        b = (a * y).astype(f32)
        return ((f32(s0) - (b * f32(s1)).astype(f32)) * y).astype(f32)

    RSQ_NR = DveOp("ANT_RSQ_NR",
                   Spec(body=(C0 - ((Src0 * Src1) * Src1) * C1) * Src1,
                        reference=_ref_rsq_nr),
                   subdim=False, uops_sha={})

    def _ref_gn2(in0, in1, s0, s1, imm2):
        raw = np.asarray(in0, f32); c = np.asarray(in1, f32)
        return np.maximum((raw - (c * c).astype(f32)).astype(f32), f32(s0))

    GN2_F = DveOp("ANT_GN2_F",
                  Spec(body=maxx(Src0 - Src1 * Src1, C0), reference=_ref_gn2),
                  subdim=False, uops_sha={})

    ops = [RSQ_F, RSQ_NR, GN2_F]
    base = dve_ops._CUSTOM_DVE_ROW_BASE + len(dve_ops.OPS)
    for i, op in enumerate(ops):
        dve_ops._SUB_OPCODE_FOR_NAME[op.name] = base + i
    for trn in ("TRN2",):
        ver = dve_ver_for(trn)
        for op in ops:
            uops = lower(op.spec, ver=ver)
            s = DveOpSpec(name=op.name, opcode=dve_ops.get_dve_sub_opcode(op.name),
                          uops=uops, rd1_en=has_src1(op.spec))
            op.uops_sha[ver] = s.sha(ver)
    dve_ops.OPS.extend(ops)
    dve_ops.CUSTOM_DVE_SPECS.update({op.name: op.spec for op in ops})
    return {op.name: op for op in ops}



def _ensure_trace_hook():
    try:
        from antenv.axon_hooks import get_axon_ntff_profile_hook
        return get_axon_ntff_profile_hook() is not None
    except ImportError:
        pass
    try:
        import sys, types
        import antenv
        from trn_agent_boot.trn_boot import _ntff_profile_via_ctypes
        mod = types.ModuleType("antenv.axon_hooks")
        _h = {}
        mod.set_axon_ntff_profile_hook = lambda h: _h.__setitem__("h", h)
        mod.get_axon_ntff_profile_hook = lambda: _h.get("h")
        sys.modules["antenv.axon_hooks"] = mod
        antenv.axon_hooks = mod
        mod.set_axon_ntff_profile_hook(
            _ntff_profile_via_ctypes("/opt/axon/libaxon_pjrt.so"))
        return True
    except Exception:
        return False





def build_program(INTERLEAVE=4, gps=(), redsplit=False, wbufs=2,
                  stagger=False, psf=2, pst=2, pss=2, dbufs=None, fold2=True, fold3=False, dup=True, qdve=False, g2dve=False, cpdve=False, smaj=False):
    from contextlib import ExitStack
    import concourse.bacc as bacc
    import concourse.mybir as mybir
    import concourse.tile as tile

    gps = frozenset(gps)
    FP = mybir.dt.float32
    BF = mybir.dt.bfloat16
    I32 = mybir.dt.int32
    AF = mybir.ActivationFunctionType
    ALU = mybir.AluOpType
    AX = mybir.AxisListType

    OPS = _register_custom_ops()
    RSQ_F, RSQ_NR, GN2_F = OPS["ANT_RSQ_F"], OPS["ANT_RSQ_NR"], OPS["ANT_GN2_F"]

    nc = bacc.Bacc()
    # x pre-transposed on host: [block, half, i, j, d]
    x_d = nc.dram_tensor("xp", (NBLK, 2, C_IN, R, D_DIM), BF,
                         kind="ExternalInput")
    # x0 rows: [block, half, j, d]
    x0_d = nc.dram_tensor("x0p", (NBLK, 2, R, D_DIM), BF, kind="ExternalInput")
    w_d = nc.dram_tensor("w_rep", (128, W), BF, kind="ExternalInput")
    id_d = nc.dram_tensor("ident2", (128, 64), BF, kind="ExternalInput")
    # output in SBUF-natural order: [block, half, o, j, d]
    out_d = nc.dram_tensor("out_p", (NBLK, 2, C_OUT, R, D_DIM), BF,
                           kind="ExternalOutput")

    ctx = ExitStack()
    with ctx:
        tc = ctx.enter_context(tile.TileContext(nc))
        const = ctx.enter_context(tc.tile_pool(name="const", bufs=1))
        xg_p = ctx.enter_context(tc.tile_pool(name="xg", bufs=NBLK))
        work = ctx.enter_context(tc.tile_pool(name="work", bufs=wbufs))
        deep = ctx.enter_context(tc.tile_pool(name="deep", bufs=dbufs or max(2, INTERLEAVE)))
        ab_p = ctx.enter_context(tc.tile_pool(name="ab", bufs=NBLK))
        red_p = ctx.enter_context(tc.tile_pool(name="red", bufs=max(2, INTERLEAVE)))
        ps_f = ctx.enter_context(tc.tile_pool(name="psf", bufs=psf, space="PSUM"))
        ps_t = ctx.enter_context(tc.tile_pool(name="pst", bufs=pst, space="PSUM"))
        ps_s = ctx.enter_context(tc.tile_pool(name="pss", bufs=pss, space="PSUM"))

        def eng(name):
            return nc.gpsimd if name in gps else nc.vector

        # ---- constants (all contiguous DMAs)
        w_g = const.tile([128, W], BF, tag="wg")
        nc.sync.dma_start(w_g[:, :], w_d[:, :])
        ident = const.tile([128, 64], BF, tag="ident")
        nc.sync.dma_start(ident[:, :], id_d[:, :])
        halfpi = const.tile([128, 1], FP, tag="halfpi")
        nc.vector.memset(halfpi[:, :], HALF_PI)

        def jbh(t, h, j):
            return t[64 * h:64 * h + 64, 64 * j:64 * j + 64]

        def b3(t):
            return t[:, :].rearrange("p (j d) -> p j d", d=64)

        def bcR(small_ap):      # [128, R] ap -> broadcast (p, j, 64)
            return small_ap.rearrange("p (j o) -> p j o", o=1) \
                .broadcast_to([128, R, 64])

        def emit_load(st):
            b = st["b"]
            X = xg_p.tile([128, W], BF, tag="xg")
            for h in (0, 1):
                nc.sync.dma_start(b3(X[64 * h:64 * h + 64, :]), x_d[b, h])
            A0 = ab_p.tile([128, W], BF, tag="a0")
            for h in (0, 1):
                nc.sync.dma_start(
                    A0[64 * h:64 * h + 64, :].rearrange("p (j d) -> p j d", d=64),
                    x0_d[b:b + 1, h].rearrange("b j d -> b j d")
                    .broadcast_to([64, R, 64]))
            tp = ps_t.tile([128, W], BF, tag="tp")
            for h in (0, 1):
                for r in range(R):
                    nc.tensor.transpose(jbh(tp, h, r), jbh(X, h, r),
                                        ident[64 * h:64 * h + 64, :])
            XT = xg_p.tile([128, W], BF, tag="xt")
            (nc.vector.tensor_copy if cpdve else nc.scalar.copy)(
                XT[:, :], tp[:, :])
            st["X"], st["XT"], st["A"] = X, XT, A0

        def quake(pool, src_ap, shape, tagp, out_dt, nr=False):
            seed = pool.tile(shape, FP, tag=tagp + "sd")
            nc.vector.tensor_scalar(seed[:, :].bitcast(I32),
                                    src_ap.bitcast(I32), 1, -1,
                                    ALU.logical_shift_right, ALU.bitwise_xor)
            rr = pool.tile(shape, out_dt, tag=tagp + "rr")
            nc.vector._custom_dve(RSQ_F, out=rr[:, :], in0=src_ap,
                                  in1=seed[:, :], s0=RSQ_C1, s1=RSQ_C2)
            if not nr:
                return rr
            rr2 = pool.tile(shape, out_dt, tag=tagp + "r2")
            nc.vector._custom_dve(RSQ_NR, out=rr2[:, :], in0=src_ap,
                                  in1=rr[:, :], s0=1.5, s1=0.5)
            return rr2

        # ---------- iteration 0: per-block D0 + small f-chain
        def emit_d0(st):
            XT = st["XT"]
            psD0 = ps_s.tile([128, R], FP, tag="d0")
            for h in (0, 1):
                for r in range(R):
                    nc.tensor.matmul(
                        psD0[64 * h:64 * h + 64, r:r + 1],
                        jbh(XT, h, r),
                        XT[64 * h:64 * h + 64, 64 * r:64 * r + 1])
            shape = [128, R]
            q0 = red_p.tile(shape, FP, tag="f0q")
            nc.scalar.activation(q0[:, :], psD0[:, :], AF.Square)
            u0 = red_p.tile(shape, FP, tag="f0u")
            nc.vector.tensor_scalar(u0[:, :], q0[:, :], -1.0, 1.0 + EPS_U,
                                    ALU.mult, ALU.add)
            rr0 = quake(red_p, u0[:, :], shape, "f0", FP)
            zs0 = red_p.tile(shape, FP, tag="f0z")
            nc.vector.tensor_tensor(zs0[:, :], psD0[:, :], rr0[:, :], ALU.mult)
            th0 = red_p.tile(shape, FP, tag="f0t")
            nc.scalar.activation(th0[:, :], zs0[:, :], AF.Arctan, scale=-1.0)
            f0 = red_p.tile(shape, BF, tag="f0v")
            nc.vector.scalar_tensor_tensor(f0[:, :], th0[:, :], HALF_PI,
                                           rr0[:, :], ALU.add, ALU.mult)
            st["f0"] = f0[:, :]

        def emit_factor(st, it):
            X, XT = st["X"], st["XT"]
            if it == 0:
                Xf = deep.tile([128, W], BF, tag="xf")
                nc.vector.tensor_tensor(b3(Xf), b3(X), bcR(st["f0"]), ALU.mult)
                st["Xf"] = Xf
                return
            AT = st["AT"]
            psD = ps_f.tile([128, W], FP, tag="mmf")
            for h in (0, 1):
                for r in range(R):
                    nc.tensor.matmul(jbh(psD, h, r), jbh(XT, h, r),
                                     jbh(AT, h, r))
            q = work.tile([128, W], FP, tag="ffq")
            if not qdve:
                nc.scalar.activation(q[:, :], psD[:, :], AF.Square)
            Dd = work.tile([128, W], BF, tag="dd")
            nc.scalar.copy(Dd[:, :], psD[:, :])
            if qdve:
                nc.vector.tensor_tensor(q[:, :], Dd[:, :], Dd[:, :], ALU.mult)
            u = work.tile([128, W], FP, tag="ffu")
            eng("u").tensor_scalar(u[:, :], q[:, :], -1.0, 1.0 + EPS_U,
                                   ALU.mult, ALU.add)
            rr = quake(work, u[:, :], [128, W], "ff", BF)
            zs = work.tile([128, W], BF, tag="zs")
            nc.vector.tensor_tensor(zs[:, :], Dd[:, :], rr[:, :], ALU.mult)
            th = work.tile([128, W], BF, tag="th")
            nc.scalar.activation(th[:, :], zs[:, :], AF.Arctan, scale=-1.0)
            thp = work.tile([128, W], BF, tag="thp")
            nc.vector.tensor_scalar(thp[:, :], th[:, :], HALF_PI, None,
                                    ALU.add)
            f = work.tile([128, W], BF, tag="fv")
            nc.vector.tensor_tensor(f[:, :], thp[:, :], rr[:, :], ALU.mult)
            S = deep.tile([128, W], BF, tag="sg")
            nc.vector.tensor_tensor(S[:, :], w_g[:, :], f[:, :], ALU.mult)
            st["S"] = S


        def emit_factor_smaj(prs, it):
            # per-block matmuls + ACT copies first
            tiles = []
            for st in prs:
                X, XT, AT = st["X"], st["XT"], st["AT"]
                psD = ps_f.tile([128, W], FP, tag="mmf")
                for h in (0, 1):
                    for r in range(R):
                        nc.tensor.matmul(jbh(psD, h, r), jbh(XT, h, r),
                                         jbh(AT, h, r))
                q = work.tile([128, W], FP, tag="ffq")
                nc.scalar.activation(q[:, :], psD[:, :], AF.Square)
                Dd = work.tile([128, W], BF, tag="dd")
                nc.scalar.copy(Dd[:, :], psD[:, :])
                tiles.append((q, Dd))
            # DVE stages interleaved across the pair
            us = []
            for q, Dd in tiles:
                u = work.tile([128, W], FP, tag="ffu")
                nc.vector.tensor_scalar(u[:, :], q[:, :], -1.0, 1.0 + EPS_U,
                                        ALU.mult, ALU.add)
                us.append(u)
            seeds = []
            for u in us:
                seed = work.tile([128, W], FP, tag="ffsd")
                nc.vector.tensor_scalar(seed[:, :].bitcast(I32),
                                        u[:, :].bitcast(I32), 1, -1,
                                        ALU.logical_shift_right,
                                        ALU.bitwise_xor)
                seeds.append(seed)
            rrs = []
            for u, seed in zip(us, seeds):
                rr = work.tile([128, W], BF, tag="ffrr")
                nc.vector._custom_dve(RSQ_F, out=rr[:, :], in0=u[:, :],
                                      in1=seed[:, :], s0=RSQ_C1, s1=RSQ_C2)
                rrs.append(rr)
            zss = []
            for (q, Dd), rr in zip(tiles, rrs):
                zs = work.tile([128, W], BF, tag="zs")
                nc.vector.tensor_tensor(zs[:, :], Dd[:, :], rr[:, :], ALU.mult)
                zss.append(zs)
            ths = []
            for zs in zss:
                th = work.tile([128, W], BF, tag="th")
                nc.scalar.activation(th[:, :], zs[:, :], AF.Arctan, scale=-1.0)
                ths.append(th)
            thps = []
            for th in ths:
                thp = work.tile([128, W], BF, tag="zs")
                nc.vector.tensor_scalar(thp[:, :], th[:, :], HALF_PI, None,
                                        ALU.add)
                thps.append(thp)
            fs = []
            for thp, rr in zip(thps, rrs):
                f = work.tile([128, W], BF, tag="th")
                nc.vector.tensor_tensor(f[:, :], thp[:, :], rr[:, :], ALU.mult)
                fs.append(f)
            for st, f in zip(prs, fs):
                S = deep.tile([128, W], BF, tag="sg")
                nc.vector.tensor_tensor(S[:, :], w_g[:, :], f[:, :], ALU.mult)
                st["S"] = S
        def emit_gmm(st, it):
            X = st["X"]
            psG = ps_f.tile([128, W], FP, tag="mmf")
            if it == 0:
                Xf = st["Xf"]
                for h in (0, 1):
                    for c in (0, 512):
                        nc.tensor.matmul(psG[64 * h:64 * h + 64, c:c + 512],
                                         w_g[64 * h:64 * h + 64, 0:64],
                                         Xf[64 * h:64 * h + 64, c:c + 512])
            else:
                S = st["S"]
                for h in (0, 1):
                    for r in range(R):
                        nc.tensor.matmul(jbh(psG, h, r), jbh(S, h, r),
                                         jbh(X, h, r))
            Gd = deep.tile([128, W], BF, tag="gd")
            nc.scalar.copy(Gd[:, :], psG[:, :])
            # write this block's prod/g2 slices now so psG frees in ACT order
            pg, idx = st["pg"], st["pgidx"]
            off = 2 * W * idx
            if g2dve:
                nc.vector.tensor_tensor(pg[:, off + W:off + 2 * W],
                                        Gd[:, :], Gd[:, :], ALU.mult)
            else:
                nc.scalar.activation(pg[:, off + W:off + 2 * W],
                                     psG[:, :], AF.Square)
            eng("prod").tensor_tensor(pg[:, off:off + W], st["A"][:, :],
                                      Gd[:, :], ALU.mult)
            st["Gd"] = Gd

        def emit_update(sts, it):
            npair = len(sts)
            pg = sts[0]["pg"]
            nred = 2 * R * npair
            red = red_p.tile([128, nred], FP, tag="red")
            # stage 1: pairwise fold at bf16 2x mode (halves reduce volume)
            fold = work.tile([128, W * npair], BF, tag="fold")
            pv = pg[:, :].rearrange("p (s two q) -> p s two q", two=2, q=32)
            nc.vector.tensor_tensor(
                fold[:, :].rearrange("p (s q) -> p s q", q=32),
                pv[:, :, 0, :], pv[:, :, 1, :], ALU.add)
            if fold2:
                fb = work.tile([128, W * npair // 2], BF, tag="fold2")
                fv2 = fold[:, :].rearrange("p (s two q) -> p s two q",
                                           two=2, q=16)
                nc.vector.tensor_tensor(
                    fb[:, :].rearrange("p (s q) -> p s q", q=16),
                    fv2[:, :, 0, :], fv2[:, :, 1, :], ALU.add)
                if fold3:
                    fc = work.tile([128, W * npair // 4], BF, tag="fold3")
                    fv3 = fb[:, :].rearrange("p (s two q) -> p s two q",
                                             two=2, q=8)
                    nc.vector.tensor_tensor(
                        fc[:, :].rearrange("p (s q) -> p s q", q=8),
                        fv3[:, :, 0, :], fv3[:, :, 1, :], ALU.add)
                    fb = fc
                    fview, qq = fb[:, :].rearrange("p (s q) -> p s q", q=8), 8
                else:
                    fview, qq = fb[:, :].rearrange("p (s q) -> p s q", q=16), 16
            else:
                fview, qq = fold[:, :].rearrange("p (s q) -> p s q", q=32), 32
            for c0 in range(0, npair, 2):
                seg = slice(2 * R * c0, 2 * R * (c0 + 2))
                nc.vector.tensor_reduce(
                    red[:, seg].rearrange("p (s j) -> p s j", j=R),
                    fview[:, 2 * R * c0:2 * R * (c0 + 2)], AX.X, ALU.add)
            # red cols: [idx][kind][j]: coefA at kind 0, gnr at kind 1
            rv = red[:, :].rearrange("p (i k j) -> p i k j", k=2, j=R)
            shape = [128, R * npair]
            coefA = red[:, :].rearrange("p (i k j) -> p (i k) j", k=2, j=R)
            # strided views
            cview = rv[:, :, 0, :]          # [128, npair, R]
            gview = rv[:, :, 1, :]
            gn2 = red_p.tile(shape, FP, tag="gn2")
            g3 = gn2[:, :].rearrange("p (i j) -> p i j", j=R)
            nc.vector._custom_dve(GN2_F, out=g3, in0=gview, in1=cview,
                                  s0=1e-30)
            rg = quake(red_p, gn2[:, :], shape, "rg", FP, nr=True)
            gn = red_p.tile(shape, FP, tag="gn")
            nc.vector.tensor_tensor(gn[:, :], gn2[:, :], rg[:, :], ALU.mult)
            cosg = red_p.tile(shape, FP, tag="cosg")
            nc.scalar.activation(cosg[:, :], gn[:, :], AF.Sin,
                                 bias=halfpi[:, 0:1])
            s1t = red_p.tile(shape, FP, tag="s1t")
            nc.scalar.activation(s1t[:, :], gn[:, :], AF.Sin)
            sc = red_p.tile(shape, FP, tag="sc")
            nc.vector.tensor_tensor(sc[:, :], s1t[:, :], rg[:, :], ALU.mult)
            t9 = red_p.tile(shape, FP, tag="t9")
            nc.vector.scalar_tensor_tensor(
                t9[:, :].rearrange("p (i j) -> p i j", j=R), sc[:, :]
                .rearrange("p (i j) -> p i j", j=R), -1.0, cview,
                ALU.mult, ALU.mult)
            alpha = red_p.tile(shape, BF, tag="alpha")
            nc.vector.tensor_tensor(alpha[:, :], cosg[:, :], t9[:, :], ALU.add)
            scb = red_p.tile(shape, BF, tag="scb")
            nc.vector.tensor_copy(scb[:, :], sc[:, :])
            if dup:
                a2 = red_p.tile([128, 2 * R * npair], BF, tag="a2")
                nc.vector.tensor_copy(
                    a2[:, :].rearrange("p (j two) -> p j two", two=2),
                    alpha[:, :].rearrange("p (j o) -> p j o", o=1)
                    .broadcast_to([128, R * npair, 2]))
                s2 = red_p.tile([128, 2 * R * npair], BF, tag="s2")
                nc.vector.tensor_copy(
                    s2[:, :].rearrange("p (j two) -> p j two", two=2),
                    scb[:, :].rearrange("p (j o) -> p j o", o=1)
                    .broadcast_to([128, R * npair, 2]))
            last = it == 2
            for idx, st in enumerate(sts):
                A, Gd = st["A"], st["Gd"]
                t1 = work.tile([128, W], BF, tag="scr1")
                t2 = work.tile([128, W], BF, tag="scr2")
                if dup:
                    def v4(t):
                        return t[:, :].rearrange(
                            "p (j o two) -> p j o two", two=2, o=32)
                    def bc4(small, i0):
                        return small[:, 2 * R * i0:2 * R * i0 + 2 * R] \
                            .rearrange("p (j o two) -> p j o two", o=1, two=2) \
                            .broadcast_to([128, R, 32, 2])
                    nc.vector.tensor_tensor(v4(t1), v4(A), bc4(a2, idx),
                                            ALU.mult)
                    nc.vector.tensor_tensor(v4(t2), v4(Gd), bc4(s2, idx),
                                            ALU.mult)
                else:
                    eng("t1").tensor_tensor(b3(t1), b3(A),
                                            bcR(alpha[:, R * idx:R * idx + R]),
                                            ALU.mult)
                    eng("t2").tensor_tensor(b3(t2), b3(Gd),
                                            bcR(scb[:, R * idx:R * idx + R]),
                                            ALU.mult)
                An = ab_p.tile([128, W], BF, tag="agf" if last else "ag")
                nc.vector.tensor_tensor(An[:, :], t1[:, :], t2[:, :], ALU.add)
                st["A"] = An
                if not last:
                    tp = ps_t.tile([128, W], BF, tag="tp")
                    for h in (0, 1):
                        for r in range(R):
                            nc.tensor.transpose(jbh(tp, h, r), jbh(An, h, r),
                                                ident[64 * h:64 * h + 64, :])
                    AT = ab_p.tile([128, W], BF, tag="at")
                    (nc.vector.tensor_copy if cpdve else nc.scalar.copy)(
                        AT[:, :], tp[:, :])
                    st["AT"] = AT
                else:
                    for h in (0, 1):
                        nc.sync.dma_start(
                            out_d[st["b"], h],
                            b3(An[64 * h:64 * h + 64, :]))

        all_sts = [{"b": b} for b in range(NBLK)]
        for st in all_sts:
            emit_load(st)
            emit_d0(st)
        for b0 in range(0, NBLK, INTERLEAVE):
            sts = all_sts[b0:b0 + INTERLEAVE]
            for it in range(3):
                pg = work.tile([128, 2 * W * len(sts)], BF, tag="pg")
                for idx, st in enumerate(sts):
                    st["pg"], st["pgidx"] = pg, idx
                if smaj and it > 0:
                    for i0 in range(0, len(sts), 2):
                        emit_factor_smaj(sts[i0:i0 + 2], it)
                    for st in sts:
                        emit_gmm(st, it)
                elif stagger:
                    n = len(sts)
                    for k in range(n + 1):
                        if k < n:
                            emit_factor(sts[k], it)
                        if k > 0:
                            emit_gmm(sts[k - 1], it)
                else:
                    for st in sts:
                        emit_factor(st, it)
                    for st in sts:
                        emit_gmm(st, it)
                emit_update(sts, it)
    nc.compile()
    return nc


def _get_program(**kw):
    key = tuple(sorted((k, tuple(v) if isinstance(v, (list, tuple, set, frozenset))
                        else v) for k, v in kw.items()))
    if key not in _COMPILED:
        _COMPILED[key] = build_program(**kw)
    return _COMPILED[key]


def kernel(x, w_raw, _trace=False, **bkw):
    import ml_dtypes
    from concourse.bass_utils import run_bass_kernel_spmd
    if _trace:
        _trace = _ensure_trace_hook()

    bf16 = ml_dtypes.bfloat16
    x = np.asarray(x, f32)
    w_raw = np.asarray(w_raw, f32)
    B, L, C_in, d = x.shape
    N = B * L
    w = np.exp((w_raw - f32(np.log(C_in))).astype(f32)).astype(f32)
    w = (w / w.sum(axis=0, keepdims=True)).astype(f32)

    xr = x.reshape(N, C_in, d)
    # per core: [NBLK, 2, R, i, d] -> transpose to [NBLK, 2, i, R, d]
    xcore = xr.reshape(N_CORES, NBLK, 2, R, C_in, d)
    xp = np.ascontiguousarray(xcore.transpose(0, 1, 2, 4, 3, 5)).astype(bf16)
    x0p = np.ascontiguousarray(xcore[:, :, :, :, 0, :]).astype(bf16)
    w_rep = np.ascontiguousarray(
        np.broadcast_to(w.T.reshape(1, 64, 1, 64), (2, 64, R, 64))
        .transpose(0, 3, 2, 1).reshape(128, W)).astype(bf16)
    # w_rep[p, (j, o)]: lower/upper halves identical, = w[i=p%64, o]
    w_rep = np.ascontiguousarray(
        np.tile(np.repeat(w[None, :, :], 1, axis=0), (2, 1, 1))  # (2,64,64)
        .reshape(2, 64, 1, 64).repeat(R, axis=2).reshape(2 * 64, R * 64)
        ).astype(bf16)
    ident2 = np.tile(np.eye(64, dtype=bf16), (2, 1))

    nc = _get_program(**bkw)
    in_maps = []
    for k in range(N_CORES):
        in_maps.append({
            "xp": xp[k],
            "x0p": x0p[k],
            "w_rep": w_rep,
            "ident2": ident2,
        })
    res = run_bass_kernel_spmd(nc, in_maps, core_ids=list(range(N_CORES)),
                               trace=_trace)
    # out_p: [NBLK, 2, o, j, d] per core -> rows
    outs = []
    for k in range(N_CORES):
        op = res.results[k]["out_p"]          # (NBLK, 2, 64, R, 64)
        outs.append(np.ascontiguousarray(op.transpose(0, 1, 3, 2, 4))
                    .reshape(ROWS_PER_CORE, C_OUT, d))
    out = np.concatenate(outs, axis=0)
    if _trace:
        kernel.last_exec_time_ns = res.exec_time_ns
        kernel.last_results = res
    return out.reshape(B, L, C_OUT, d).astype(f32)


# revision 11
# speedup vs baseline: 1.1353x; 1.0974x over previous
"""Trainium2 Bass kernel v3 for nn_MfdFC. See kernel_v2 docstring for math.

v3 over v2:
- host pre-transposes x into per-(block,half) [i, j, d] layout and w into the
  replicated [128, 1024] SBUF image -> all input DMAs are contiguous; the
  output is written in SBUF-natural [o, j, d] order and re-transposed on host.
- the per-block [128,16] "smalls" pipelines (iter-0 f-chain, update-phase
  cos/sin/rsqrt chain) run once per interleave-PAIR on [128,32] tiles, and
  the coefA/|G|^2 reductions of a pair are fused into ONE 4096-wide reduce.
- optional GPSIMD offload for selected elementwise passes (t2, prod).
"""
import math
import numpy as np

f32 = np.float32

C_IN = 64
C_OUT = 64
D_DIM = 64
ROWS_PER_CORE = 128
N_CORES = 8
R = 16
NBLK = 4
W = 64 * R
RSQ_C1 = 1.7584694439735017e-30
RSQ_C2 = -2.755803843779718e-20
HALF_PI = float(f32(math.pi / 2.0))
EPS_U = float(f32(2.0 ** -22))

_COMPILED = {}

def _register_custom_ops():
    import concourse.dve_ops as dve_ops
    from concourse.dve_ops import DveOp
    from concourse.dve_spec import (
        Spec, Src0, Src1, C0, C1, lower, maxx, _has_src1 as has_src1,
    )
    from concourse.dve_uop import DveOpSpec
    from concourse.dve_table_gen import dve_ver_for

    if "ANT_RSQ_F" in dve_ops._SUB_OPCODE_FOR_NAME:
        return {n: op for n, op in ((o.name, o) for o in dve_ops.OPS)
                if n.startswith("ANT_")}

    def _ref_rsq_f(in0, in1, s0, s1, imm2):
        u = np.asarray(in0, f32)
        nt = np.asarray(in1, f32)
        m1 = (nt * f32(s0)).astype(f32)
        m2 = (m1 * nt).astype(f32)
        m3 = (m2 * f32(s0)).astype(f32)
        t = (m3 * u).astype(f32)
        return ((t + f32(s1)) * nt).astype(f32)

    _m1 = Src1 * C0
    _m3 = (_m1 * Src1) * C0
    RSQ_F = DveOp("ANT_RSQ_F",
                  Spec(body=((_m3 * Src0) + C1) * Src1, reference=_ref_rsq_f),
                  subdim=False, uops_sha={})

    def _ref_rsq_nr(in0, in1, s0, s1, imm2):
        u = np.asarray(in0, f32); y = np.asarray(in1, f32)
        a = (u * y).astype(f32)
        b = (a * y).astype(f32)
        return ((f32(s0) - (b * f32(s1)).astype(f32)) * y).astype(f32)

    RSQ_NR = DveOp("ANT_RSQ_NR",
                   Spec(body=(C0 - ((Src0 * Src1) * Src1) * C1) * Src1,
                        reference=_ref_rsq_nr),
                   subdim=False, uops_sha={})

    def _ref_gn2(in0, in1, s0, s1, imm2):
        raw = np.asarray(in0, f32); c = np.asarray(in1, f32)
        return np.maximum((raw - (c * c).astype(f32)).astype(f32), f32(s0))

    GN2_F = DveOp("ANT_GN2_F",
                  Spec(body=maxx(Src0 - Src1 * Src1, C0), reference=_ref_gn2),
                  subdim=False, uops_sha={})

    ops = [RSQ_F, RSQ_NR, GN2_F]
    base = dve_ops._CUSTOM_DVE_ROW_BASE + len(dve_ops.OPS)
    for i, op in enumerate(ops):
        dve_ops._SUB_OPCODE_FOR_NAME[op.name] = base + i
    for trn in ("TRN2",):
        ver = dve_ver_for(trn)
        for op in ops:
            uops = lower(op.spec, ver=ver)
            s = DveOpSpec(name=op.name, opcode=dve_ops.get_dve_sub_opcode(op.name),
                          uops=uops, rd1_en=has_src1(op.spec))
            op.uops_sha[ver] = s.sha(ver)
    dve_ops.OPS.extend(ops)
    dve_ops.CUSTOM_DVE_SPECS.update({op.name: op.spec for op in ops})
    return {op.name: op for op in ops}



def _ensure_trace_hook():
    try:
        from antenv.axon_hooks import get_axon_ntff_profile_hook
        return get_axon_ntff_profile_hook() is not None
    except ImportError:
        pass
    try:
        import sys, types
        import antenv
        from trn_agent_boot.trn_boot import _ntff_profile_via_ctypes
        mod = types.ModuleType("antenv.axon_hooks")
        _h = {}
        mod.set_axon_ntff_profile_hook = lambda h: _h.__setitem__("h", h)
        mod.get_axon_ntff_profile_hook = lambda: _h.get("h")
        sys.modules["antenv.axon_hooks"] = mod
        antenv.axon_hooks = mod
        mod.set_axon_ntff_profile_hook(
            _ntff_profile_via_ctypes("/opt/axon/libaxon_pjrt.so"))
        return True
    except Exception:
        return False





def build_program(INTERLEAVE=4, gps=(), redsplit=False, wbufs=2,
                  stagger=False, psf=2, pst=2, pss=2, dbufs=None, fold2=True, fold3=False, dup=True, qdve=False, g2dve=False, cpdve=False, smaj=False, cpdma=False, nodd=False, pairup=True, upg=1, d0dve=True):
    from contextlib import ExitStack
    import concourse.bacc as bacc
    import concourse.mybir as mybir
    import concourse.tile as tile

    gps = frozenset(gps)
    FP = mybir.dt.float32
    BF = mybir.dt.bfloat16
    I32 = mybir.dt.int32
    AF = mybir.ActivationFunctionType
    ALU = mybir.AluOpType
    AX = mybir.AxisListType

    OPS = _register_custom_ops()
    RSQ_F, RSQ_NR, GN2_F = OPS["ANT_RSQ_F"], OPS["ANT_RSQ_NR"], OPS["ANT_GN2_F"]

    nc = bacc.Bacc()
    # x pre-transposed on host: [block, half, i, j, d]
    x_d = nc.dram_tensor("xp", (NBLK, 2, C_IN, R, D_DIM), BF,
                         kind="ExternalInput")
    # x0 rows: [block, half, j, d]
    x0_d = nc.dram_tensor("x0p", (NBLK, 2, R, D_DIM), BF, kind="ExternalInput")
    w_d = nc.dram_tensor("w_rep", (128, W), BF, kind="ExternalInput")
    id_d = nc.dram_tensor("ident2", (128, 64), BF, kind="ExternalInput")
    # output in SBUF-natural order: [block, half, o, j, d]
    out_d = nc.dram_tensor("out_p", (NBLK, 2, C_OUT, R, D_DIM), BF,
                           kind="ExternalOutput")

    ctx = ExitStack()
    with ctx:
        tc = ctx.enter_context(tile.TileContext(nc))
        const = ctx.enter_context(tc.tile_pool(name="const", bufs=1))
        xg_p = ctx.enter_context(tc.tile_pool(name="xg", bufs=NBLK))
        work = ctx.enter_context(tc.tile_pool(name="work", bufs=wbufs))
        deep = ctx.enter_context(tc.tile_pool(name="deep", bufs=dbufs or max(2, INTERLEAVE)))
        ab_p = ctx.enter_context(tc.tile_pool(name="ab", bufs=NBLK))
        red_p = ctx.enter_context(tc.tile_pool(name="red", bufs=max(2, INTERLEAVE)))
        ps_f = ctx.enter_context(tc.tile_pool(name="psf", bufs=psf, space="PSUM"))
        ps_t = ctx.enter_context(tc.tile_pool(name="pst", bufs=pst, space="PSUM"))
        ps_s = ctx.enter_context(tc.tile_pool(name="pss", bufs=pss, space="PSUM"))

        def eng(name):
            return nc.gpsimd if name in gps else nc.vector

        # ---- constants (all contiguous DMAs)
        w_g = const.tile([128, W], BF, tag="wg")
        nc.sync.dma_start(w_g[:, :], w_d[:, :])
        ident = const.tile([128, 64], BF, tag="ident")
        nc.sync.dma_start(ident[:, :], id_d[:, :])
        halfpi = const.tile([128, 1], FP, tag="halfpi")
        nc.vector.memset(halfpi[:, :], HALF_PI)

        def jbh(t, h, j):
            return t[64 * h:64 * h + 64, 64 * j:64 * j + 64]

        def b3(t):
            return t[:, :].rearrange("p (j d) -> p j d", d=64)

        def bcR(small_ap):      # [128, R] ap -> broadcast (p, j, 64)
            return small_ap.rearrange("p (j o) -> p j o", o=1) \
                .broadcast_to([128, R, 64])

        def emit_load(st):
            b = st["b"]
            X = xg_p.tile([128, W], BF, tag="xg")
            for h in (0, 1):
                nc.sync.dma_start(b3(X[64 * h:64 * h + 64, :]), x_d[b, h])
            A0 = ab_p.tile([128, W], BF, tag="a0")
            for h in (0, 1):
                nc.sync.dma_start(
                    A0[64 * h:64 * h + 64, :].rearrange("p (j d) -> p j d", d=64),
                    x0_d[b:b + 1, h].rearrange("b j d -> b j d")
                    .broadcast_to([64, R, 64]))
            st["X"], st["A"] = X, A0
            if not d0dve:
                emit_xt(st)

        def emit_xt(st):
            X = st["X"]
            tp = ps_t.tile([128, W], BF, tag="tp")
            for h in (0, 1):
                for r in range(R):
                    nc.tensor.transpose(jbh(tp, h, r), jbh(X, h, r),
                                        ident[64 * h:64 * h + 64, :])
            XT = xg_p.tile([128, W], BF, tag="xt")
            if cpdma:
                nc.sync.dma_start(XT[:, :], tp[:, :])
            else:
                (nc.vector.tensor_copy if cpdve else nc.scalar.copy)(
                    XT[:, :], tp[:, :])
            st["XT"] = XT

        def quake(pool, src_ap, shape, tagp, out_dt, nr=False):
            seed = pool.tile(shape, FP, tag=tagp + "sd")
            nc.vector.tensor_scalar(seed[:, :].bitcast(I32),
                                    src_ap.bitcast(I32), 1, -1,
                                    ALU.logical_shift_right, ALU.bitwise_xor)
            rr = pool.tile(shape, out_dt, tag=tagp + "rr")
            nc.vector._custom_dve(RSQ_F, out=rr[:, :], in0=src_ap,
                                  in1=seed[:, :], s0=RSQ_C1, s1=RSQ_C2)
            if not nr:
                return rr
            rr2 = pool.tile(shape, out_dt, tag=tagp + "r2")
            nc.vector._custom_dve(RSQ_NR, out=rr2[:, :], in0=src_ap,
                                  in1=rr[:, :], s0=1.5, s1=0.5)
            return rr2

        # ---------- iteration 0: per-block D0 + small f-chain
        def emit_d0(st):
            shape = [128, R]
            if d0dve:
                # D0 via DVE: no XT dependency -> starts right after the DMAs
                X, A0 = st["X"], st["A"]
                prod0 = deep.tile([128, W], BF, tag="xf")
                nc.vector.tensor_tensor(prod0[:, :], X[:, :], A0[:, :],
                                        ALU.mult)
                D0 = red_p.tile(shape, FP, tag="f0d")
                nc.vector.tensor_reduce(D0[:, :], b3(prod0), AX.X, ALU.add)
                D0ap = D0[:, :]
                q0 = red_p.tile(shape, FP, tag="f0q")
                nc.vector.tensor_tensor(q0[:, :], D0ap, D0ap, ALU.mult)
            else:
                XT = st["XT"]
                psD0 = ps_s.tile([128, R], FP, tag="d0")
                for h in (0, 1):
                    for r in range(R):
                        nc.tensor.matmul(
                            psD0[64 * h:64 * h + 64, r:r + 1],
                            jbh(XT, h, r),
                            XT[64 * h:64 * h + 64, 64 * r:64 * r + 1])
                D0ap = psD0[:, :]
                q0 = red_p.tile(shape, FP, tag="f0q")
                nc.scalar.activation(q0[:, :], D0ap, AF.Square)
            u0 = red_p.tile(shape, FP, tag="f0u")
            nc.vector.tensor_scalar(u0[:, :], q0[:, :], -1.0, 1.0 + EPS_U,
                                    ALU.mult, ALU.add)
            rr0 = quake(red_p, u0[:, :], shape, "f0", FP)
            zs0 = red_p.tile(shape, FP, tag="f0z")
            nc.vector.tensor_tensor(zs0[:, :], D0ap, rr0[:, :], ALU.mult)
            th0 = red_p.tile(shape, FP, tag="f0t")
            nc.scalar.activation(th0[:, :], zs0[:, :], AF.Arctan, scale=-1.0)
            f0 = red_p.tile(shape, BF, tag="f0v")
            nc.vector.scalar_tensor_tensor(f0[:, :], th0[:, :], HALF_PI,
                                           rr0[:, :], ALU.add, ALU.mult)
            st["f0"] = f0[:, :]

        def emit_factor(st, it):
            X, XT = st["X"], st["XT"]
            if it == 0:
                Xf = deep.tile([128, W], BF, tag="xf")
                nc.vector.tensor_tensor(b3(Xf), b3(X), bcR(st["f0"]), ALU.mult)
                st["Xf"] = Xf
                return
            AT = st["AT"]
            psD = ps_f.tile([128, W], FP, tag="mmf")
            for h in (0, 1):
                for r in range(R):
                    nc.tensor.matmul(jbh(psD, h, r), jbh(XT, h, r),
                                     jbh(AT, h, r))
            q = work.tile([128, W], FP, tag="ffq")
            if not qdve:
                nc.scalar.activation(q[:, :], psD[:, :], AF.Square)
            if nodd:
                Dd = psD
            else:
                Dd = work.tile([128, W], BF, tag="dd")
                nc.scalar.copy(Dd[:, :], psD[:, :])
            if qdve:
                nc.vector.tensor_tensor(q[:, :], Dd[:, :], Dd[:, :], ALU.mult)
            u = work.tile([128, W], FP, tag="ffu")
            eng("u").tensor_scalar(u[:, :], q[:, :], -1.0, 1.0 + EPS_U,
                                   ALU.mult, ALU.add)
            rr = quake(work, u[:, :], [128, W], "ff", BF)
            zs = work.tile([128, W], BF, tag="zs")
            nc.vector.tensor_tensor(zs[:, :], Dd[:, :], rr[:, :], ALU.mult)
            th = work.tile([128, W], BF, tag="th")
            nc.scalar.activation(th[:, :], zs[:, :], AF.Arctan, scale=-1.0)
            thp = work.tile([128, W], BF, tag="thp")
            nc.vector.tensor_scalar(thp[:, :], th[:, :], HALF_PI, None,
                                    ALU.add)
            f = work.tile([128, W], BF, tag="fv")
            nc.vector.tensor_tensor(f[:, :], thp[:, :], rr[:, :], ALU.mult)
            S = deep.tile([128, W], BF, tag="sg")
            nc.vector.tensor_tensor(S[:, :], w_g[:, :], f[:, :], ALU.mult)
            st["S"] = S


        def emit_factor_smaj(prs, it):
            # per-block matmuls + ACT copies first
            tiles = []
            for st in prs:
                X, XT, AT = st["X"], st["XT"], st["AT"]
                psD = ps_f.tile([128, W], FP, tag="mmf")
                for h in (0, 1):
                    for r in range(R):
                        nc.tensor.matmul(jbh(psD, h, r), jbh(XT, h, r),
                                         jbh(AT, h, r))
                q = work.tile([128, W], FP, tag="ffq")
                nc.scalar.activation(q[:, :], psD[:, :], AF.Square)
                Dd = work.tile([128, W], BF, tag="dd")
                nc.scalar.copy(Dd[:, :], psD[:, :])
                tiles.append((q, Dd))
            # DVE stages interleaved across the pair
            us = []
            for q, Dd in tiles:
                u = work.tile([128, W], FP, tag="ffu")
                nc.vector.tensor_scalar(u[:, :], q[:, :], -1.0, 1.0 + EPS_U,
                                        ALU.mult, ALU.add)
                us.append(u)
            seeds = []
            for u in us:
                seed = work.tile([128, W], FP, tag="ffsd")
                nc.vector.tensor_scalar(seed[:, :].bitcast(I32),
                                        u[:, :].bitcast(I32), 1, -1,
                                        ALU.logical_shift_right,
                                        ALU.bitwise_xor)
                seeds.append(seed)
            rrs = []
            for u, seed in zip(us, seeds):
                rr = work.tile([128, W], BF, tag="ffrr")
                nc.vector._custom_dve(RSQ_F, out=rr[:, :], in0=u[:, :],
                                      in1=seed[:, :], s0=RSQ_C1, s1=RSQ_C2)
                rrs.append(rr)
            zss = []
            for (q, Dd), rr in zip(tiles, rrs):
                zs = work.tile([128, W], BF, tag="zs")
                nc.vector.tensor_tensor(zs[:, :], Dd[:, :], rr[:, :], ALU.mult)
                zss.append(zs)
            ths = []
            for zs in zss:
                th = work.tile([128, W], BF, tag="th")
                nc.scalar.activation(th[:, :], zs[:, :], AF.Arctan, scale=-1.0)
                ths.append(th)
            thps = []
            for th in ths:
                thp = work.tile([128, W], BF, tag="zs")
                nc.vector.tensor_scalar(thp[:, :], th[:, :], HALF_PI, None,
                                        ALU.add)
                thps.append(thp)
            fs = []
            for thp, rr in zip(thps, rrs):
                f = work.tile([128, W], BF, tag="th")
                nc.vector.tensor_tensor(f[:, :], thp[:, :], rr[:, :], ALU.mult)
                fs.append(f)
            for st, f in zip(prs, fs):
                S = deep.tile([128, W], BF, tag="sg")
                nc.vector.tensor_tensor(S[:, :], w_g[:, :], f[:, :], ALU.mult)
                st["S"] = S
        def emit_gmm(st, it):
            X = st["X"]
            psG = ps_f.tile([128, W], FP, tag="mmf")
            if it == 0:
                Xf = st["Xf"]
                for h in (0, 1):
                    for c in (0, 512):
                        nc.tensor.matmul(psG[64 * h:64 * h + 64, c:c + 512],
                                         w_g[64 * h:64 * h + 64, 0:64],
                                         Xf[64 * h:64 * h + 64, c:c + 512])
            else:
                S = st["S"]
                for h in (0, 1):
                    for r in range(R):
                        nc.tensor.matmul(jbh(psG, h, r), jbh(S, h, r),
                                         jbh(X, h, r))
            Gd = deep.tile([128, W], BF, tag="gd")
            nc.scalar.copy(Gd[:, :], psG[:, :])
            # write this block's prod/g2 slices now so psG frees in ACT order
            pg, idx = st["pg"], st["pgidx"]
            off = 2 * W * idx
            if g2dve:
                nc.vector.tensor_tensor(pg[:, off + W:off + 2 * W],
                                        Gd[:, :], Gd[:, :], ALU.mult)
            else:
                nc.scalar.activation(pg[:, off + W:off + 2 * W],
                                     psG[:, :], AF.Square)
            eng("prod").tensor_tensor(pg[:, off:off + W], st["A"][:, :],
                                      Gd[:, :], ALU.mult)
            st["Gd"] = Gd

        def emit_update(sts, it):
            npair = len(sts)
            pg = sts[0]["pg"]
            nred = 2 * R * npair
            red = red_p.tile([128, nred], FP, tag="red")
            # stage 1: pairwise fold at bf16 2x mode (halves reduce volume)
            fold = work.tile([128, W * npair], BF, tag="fold")
            pv = pg[:, :].rearrange("p (s two q) -> p s two q", two=2, q=32)
            nc.vector.tensor_tensor(
                fold[:, :].rearrange("p (s q) -> p s q", q=32),
                pv[:, :, 0, :], pv[:, :, 1, :], ALU.add)
            if fold2:
                fb = work.tile([128, W * npair // 2], BF, tag="fold2")
                fv2 = fold[:, :].rearrange("p (s two q) -> p s two q",
                                           two=2, q=16)
                nc.vector.tensor_tensor(
                    fb[:, :].rearrange("p (s q) -> p s q", q=16),
                    fv2[:, :, 0, :], fv2[:, :, 1, :], ALU.add)
                if fold3:
                    fc = work.tile([128, W * npair // 4], BF, tag="fold3")
                    fv3 = fb[:, :].rearrange("p (s two q) -> p s two q",
                                             two=2, q=8)
                    nc.vector.tensor_tensor(
                        fc[:, :].rearrange("p (s q) -> p s q", q=8),
                        fv3[:, :, 0, :], fv3[:, :, 1, :], ALU.add)
                    fb = fc
                    fview, qq = fb[:, :].rearrange("p (s q) -> p s q", q=8), 8
                else:
                    fview, qq = fb[:, :].rearrange("p (s q) -> p s q", q=16), 16
            else:
                fview, qq = fold[:, :].rearrange("p (s q) -> p s q", q=32), 32
            for c0 in range(0, npair, 2):
                hi = min(c0 + 2, npair)
                seg = slice(2 * R * c0, 2 * R * hi)
                nc.vector.tensor_reduce(
                    red[:, seg].rearrange("p (s j) -> p s j", j=R),
                    fview[:, 2 * R * c0:2 * R * hi], AX.X, ALU.add)
            # red cols: [idx][kind][j]: coefA at kind 0, gnr at kind 1
            rv = red[:, :].rearrange("p (i k j) -> p i k j", k=2, j=R)
            shape = [128, R * npair]
            coefA = red[:, :].rearrange("p (i k j) -> p (i k) j", k=2, j=R)
            # strided views
            cview = rv[:, :, 0, :]          # [128, npair, R]
            gview = rv[:, :, 1, :]
            gn2 = red_p.tile(shape, FP, tag="gn2")
            g3 = gn2[:, :].rearrange("p (i j) -> p i j", j=R)
            nc.vector._custom_dve(GN2_F, out=g3, in0=gview, in1=cview,
                                  s0=1e-30)
            rg = quake(red_p, gn2[:, :], shape, "rg", FP, nr=True)
            gn = red_p.tile(shape, FP, tag="gn")
            nc.vector.tensor_tensor(gn[:, :], gn2[:, :], rg[:, :], ALU.mult)
            cosg = red_p.tile(shape, FP, tag="cosg")
            nc.scalar.activation(cosg[:, :], gn[:, :], AF.Sin,
                                 bias=halfpi[:, 0:1])
            s1t = red_p.tile(shape, FP, tag="s1t")
            nc.scalar.activation(s1t[:, :], gn[:, :], AF.Sin)
            sc = red_p.tile(shape, FP, tag="sc")
            nc.vector.tensor_tensor(sc[:, :], s1t[:, :], rg[:, :], ALU.mult)
            t9 = red_p.tile(shape, FP, tag="t9")
            nc.vector.scalar_tensor_tensor(
                t9[:, :].rearrange("p (i j) -> p i j", j=R), sc[:, :]
                .rearrange("p (i j) -> p i j", j=R), -1.0, cview,
                ALU.mult, ALU.mult)
            alpha = red_p.tile(shape, BF, tag="alpha")
            nc.vector.tensor_tensor(alpha[:, :], cosg[:, :], t9[:, :], ALU.add)
            scb = red_p.tile(shape, BF, tag="scb")
            nc.vector.tensor_copy(scb[:, :], sc[:, :])
            if dup:
                a2 = red_p.tile([128, 2 * R * npair], BF, tag="a2")
                nc.vector.tensor_copy(
                    a2[:, :].rearrange("p (j two) -> p j two", two=2),
                    alpha[:, :].rearrange("p (j o) -> p j o", o=1)
                    .broadcast_to([128, R * npair, 2]))
                s2 = red_p.tile([128, 2 * R * npair], BF, tag="s2")
                nc.vector.tensor_copy(
                    s2[:, :].rearrange("p (j two) -> p j two", two=2),
                    scb[:, :].rearrange("p (j o) -> p j o", o=1)
                    .broadcast_to([128, R * npair, 2]))
            last = it == 2
            for idx, st in enumerate(sts):
                A, Gd = st["A"], st["Gd"]
                t1 = work.tile([128, W], BF, tag="scr1")
                t2 = work.tile([128, W], BF, tag="scr2")
                if dup:
                    def v4(t):
                        return t[:, :].rearrange(
                            "p (j o two) -> p j o two", two=2, o=32)
                    def bc4(small, i0):
                        return small[:, 2 * R * i0:2 * R * i0 + 2 * R] \
                            .rearrange("p (j o two) -> p j o two", o=1, two=2) \
                            .broadcast_to([128, R, 32, 2])
                    nc.vector.tensor_tensor(v4(t1), v4(A), bc4(a2, idx),
                                            ALU.mult)
                    nc.vector.tensor_tensor(v4(t2), v4(Gd), bc4(s2, idx),
                                            ALU.mult)
                else:
                    eng("t1").tensor_tensor(b3(t1), b3(A),
                                            bcR(alpha[:, R * idx:R * idx + R]),
                                            ALU.mult)
                    eng("t2").tensor_tensor(b3(t2), b3(Gd),
                                            bcR(scb[:, R * idx:R * idx + R]),
                                            ALU.mult)
                An = ab_p.tile([128, W], BF, tag="agf" if last else "ag")
                nc.vector.tensor_tensor(An[:, :], t1[:, :], t2[:, :], ALU.add)
                st["A"] = An
                if not last:
                    tp = ps_t.tile([128, W], BF, tag="tp")
                    for h in (0, 1):
                        for r in range(R):
                            nc.tensor.transpose(jbh(tp, h, r), jbh(An, h, r),
                                                ident[64 * h:64 * h + 64, :])
                    AT = ab_p.tile([128, W], BF, tag="at")
                    if cpdma:
                        nc.sync.dma_start(AT[:, :], tp[:, :])
                    else:
                        (nc.vector.tensor_copy if cpdve else nc.scalar.copy)(
                            AT[:, :], tp[:, :])
                    st["AT"] = AT
                else:
                    for h in (0, 1):
                        nc.sync.dma_start(
                            out_d[st["b"], h],
                            b3(An[64 * h:64 * h + 64, :]))

        all_sts = [{"b": b} for b in range(NBLK)]
        for st in all_sts:
            emit_load(st)
            emit_d0(st)
        if d0dve:
            for st in all_sts:
                emit_xt(st)
        for b0 in range(0, NBLK, INTERLEAVE):
            sts = all_sts[b0:b0 + INTERLEAVE]
            for it in range(3):
                if pairup:
                    for st in sts:
                        emit_factor(st, it)
                    for i0 in range(0, len(sts), upg):
                        pr = sts[i0:i0 + upg]
                        pgp = work.tile([128, 2 * W * len(pr)], BF, tag="pg")
                        for idx, st in enumerate(pr):
                            st["pg"], st["pgidx"] = pgp, idx
                            emit_gmm(st, it)
                        emit_update(pr, it)
                    continue
                pg = work.tile([128, 2 * W * len(sts)], BF, tag="pg")
                for idx, st in enumerate(sts):
                    st["pg"], st["pgidx"] = pg, idx
                if smaj and it > 0:
                    for i0 in range(0, len(sts), 2):
                        emit_factor_smaj(sts[i0:i0 + 2], it)
                    for st in sts:
                        emit_gmm(st, it)
                elif stagger:
                    n = len(sts)
                    for k in range(n + 1):
                        if k < n:
                            emit_factor(sts[k], it)
                        if k > 0:
                            emit_gmm(sts[k - 1], it)
                else:
                    for st in sts:
                        emit_factor(st, it)
                    for st in sts:
                        emit_gmm(st, it)
                emit_update(sts, it)
    nc.compile()
    return nc


def _get_program(**kw):
    key = tuple(sorted((k, tuple(v) if isinstance(v, (list, tuple, set, frozenset))
                        else v) for k, v in kw.items()))
    if key not in _COMPILED:
        _COMPILED[key] = build_program(**kw)
    return _COMPILED[key]


def kernel(x, w_raw, _trace=False, **bkw):
    import ml_dtypes
    from concourse.bass_utils import run_bass_kernel_spmd
    if _trace:
        _trace = _ensure_trace_hook()

    bf16 = ml_dtypes.bfloat16
    x = np.asarray(x, f32)
    w_raw = np.asarray(w_raw, f32)
    B, L, C_in, d = x.shape
    N = B * L
    w = np.exp((w_raw - f32(np.log(C_in))).astype(f32)).astype(f32)
    w = (w / w.sum(axis=0, keepdims=True)).astype(f32)

    xr = x.reshape(N, C_in, d)
    # per core: [NBLK, 2, R, i, d] -> transpose to [NBLK, 2, i, R, d]
    xcore = xr.reshape(N_CORES, NBLK, 2, R, C_in, d)
    xp = np.ascontiguousarray(xcore.transpose(0, 1, 2, 4, 3, 5)).astype(bf16)
    x0p = np.ascontiguousarray(xcore[:, :, :, :, 0, :]).astype(bf16)
    w_rep = np.ascontiguousarray(
        np.broadcast_to(w.T.reshape(1, 64, 1, 64), (2, 64, R, 64))
        .transpose(0, 3, 2, 1).reshape(128, W)).astype(bf16)
    # w_rep[p, (j, o)]: lower/upper halves identical, = w[i=p%64, o]
    w_rep = np.ascontiguousarray(
        np.tile(np.repeat(w[None, :, :], 1, axis=0), (2, 1, 1))  # (2,64,64)
        .reshape(2, 64, 1, 64).repeat(R, axis=2).reshape(2 * 64, R * 64)
        ).astype(bf16)
    ident2 = np.tile(np.eye(64, dtype=bf16), (2, 1))

    nc = _get_program(**bkw)
    in_maps = []
    for k in range(N_CORES):
        in_maps.append({
            "xp": xp[k],
            "x0p": x0p[k],
            "w_rep": w_rep,
            "ident2": ident2,
        })
    res = run_bass_kernel_spmd(nc, in_maps, core_ids=list(range(N_CORES)),
                               trace=_trace)
    # out_p: [NBLK, 2, o, j, d] per core -> rows
    outs = []
    for k in range(N_CORES):
        op = res.results[k]["out_p"]          # (NBLK, 2, 64, R, 64)
        outs.append(np.ascontiguousarray(op.transpose(0, 1, 3, 2, 4))
                    .reshape(ROWS_PER_CORE, C_OUT, d))
    out = np.concatenate(outs, axis=0)
    if _trace:
        kernel.last_exec_time_ns = res.exec_time_ns
        kernel.last_results = res
    return out.reshape(B, L, C_OUT, d).astype(f32)


# revision 12
# speedup vs baseline: 1.1425x; 1.0063x over previous
"""Trainium2 Bass kernel for nn_MfdFC (spherical weighted-Frechet-mean).

Math per row n (N=1024, 128 rows/core): w = col-softmax(w_raw); a(o) <- x0;
3 iterations of  D = <a_o, x_i>;  f = (pi/2 + arctan(-D*rr))*rr with
rr = rsqrt(1+eps-D^2) (quake rsqrt on DVE, arctan on ACT);  S = w^T。f;
G = S @ X;  c = sum_d A。G;  gn = sqrt(sum G^2 - c^2);
a <- (cos gn - sinc(gn) c) a + sinc(gn) G.

Design: all matmuls bf16 (4x PE rate); elementwise bf16 in DVE 2x packed
mode. 128 rows/core as 4 blocks of 32, halves stacked at PSUM partitions
0-63/64-127 (PE writes upper partitions directly); all four blocks
pipelined (INTERLEAVE=4) with deep-rotation pools so tile reuse cannot
cycle against the in-order engine queues. The update phase runs PER BLOCK
(upg=1) so iteration boundaries stagger: while one block round-trips
through AT-transpose -> D-matmul -> Square, the other blocks' update work
keeps the DVE fed. Iteration-0 D0 is computed ON the DVE (X。A0 + reduce,
no transpose needed) and the XT transposes are deferred under iteration-0
compute, shrinking the startup stall. Host pre-transposes x / pre-
replicates w (contiguous DMAs); output returns bf16, cast on host. The
coefA/|G|^2 reductions run as a tree (two bf16 pairwise folds at 2x rate,
then a 1x tensor_reduce on 16x less data). The alpha/sinc scalars are
stored duplicated in adjacent pairs so their d-broadcast APs have
stride-1 2-element inner runs, qualifying t1/t2 for DVE 2x packed mode.
q (Square) is emitted before the D-copy so ACT serves the longer DVE
rsqrt chain first.
"""
import math
import numpy as np

f32 = np.float32

C_IN = 64
C_OUT = 64
D_DIM = 64
ROWS_PER_CORE = 128
N_CORES = 8
R = 16
NBLK = 4
W = 64 * R
RSQ_C1 = 1.7584694439735017e-30
RSQ_C2 = -2.755803843779718e-20
HALF_PI = float(f32(math.pi / 2.0))
EPS_U = float(f32(2.0 ** -22))

_COMPILED = {}

def _register_custom_ops():
    import concourse.dve_ops as dve_ops
    from concourse.dve_ops import DveOp
    from concourse.dve_spec import (
        Spec, Src0, Src1, C0, C1, lower, maxx, _has_src1 as has_src1,
    )
    from concourse.dve_uop import DveOpSpec
    from concourse.dve_table_gen import dve_ver_for

    if "ANT_RSQ_F" in dve_ops._SUB_OPCODE_FOR_NAME:
        return {n: op for n, op in ((o.name, o) for o in dve_ops.OPS)
                if n.startswith("ANT_")}

    def _ref_rsq_f(in0, in1, s0, s1, imm2):
        u = np.asarray(in0, f32)
        nt = np.asarray(in1, f32)
        m1 = (nt * f32(s0)).astype(f32)
        m2 = (m1 * nt).astype(f32)
        m3 = (m2 * f32(s0)).astype(f32)
        t = (m3 * u).astype(f32)
        return ((t + f32(s1)) * nt).astype(f32)

    _m1 = Src1 * C0
    _m3 = (_m1 * Src1) * C0
    RSQ_F = DveOp("ANT_RSQ_F",
                  Spec(body=((_m3 * Src0) + C1) * Src1, reference=_ref_rsq_f),
                  subdim=False, uops_sha={})

    def _ref_rsq_nr(in0, in1, s0, s1, imm2):
        u = np.asarray(in0, f32); y = np.asarray(in1, f32)
        a = (u * y).astype(f32)
        b = (a * y).astype(f32)
        return ((f32(s0) - (b * f32(s1)).astype(f32)) * y).astype(f32)

    RSQ_NR = DveOp("ANT_RSQ_NR",
                   Spec(body=(C0 - ((Src0 * Src1) * Src1) * C1) * Src1,
                        reference=_ref_rsq_nr),
                   subdim=False, uops_sha={})

    def _ref_gn2(in0, in1, s0, s1, imm2):
        raw = np.asarray(in0, f32); c = np.asarray(in1, f32)
        return np.maximum((raw - (c * c).astype(f32)).astype(f32), f32(s0))

    GN2_F = DveOp("ANT_GN2_F",
                  Spec(body=maxx(Src0 - Src1 * Src1, C0), reference=_ref_gn2),
                  subdim=False, uops_sha={})

    ops = [RSQ_F, RSQ_NR, GN2_F]
    base = dve_ops._CUSTOM_DVE_ROW_BASE + len(dve_ops.OPS)
    for i, op in enumerate(ops):
        dve_ops._SUB_OPCODE_FOR_NAME[op.name] = base + i
    for trn in ("TRN2",):
        ver = dve_ver_for(trn)
        for op in ops:
            uops = lower(op.spec, ver=ver)
            s = DveOpSpec(name=op.name, opcode=dve_ops.get_dve_sub_opcode(op.name),
                          uops=uops, rd1_en=has_src1(op.spec))
            op.uops_sha[ver] = s.sha(ver)
    dve_ops.OPS.extend(ops)
    dve_ops.CUSTOM_DVE_SPECS.update({op.name: op.spec for op in ops})
    return {op.name: op for op in ops}



def _ensure_trace_hook():
    try:
        from antenv.axon_hooks import get_axon_ntff_profile_hook
        return get_axon_ntff_profile_hook() is not None
    except ImportError:
        pass
    try:
        import sys, types
        import antenv
        from trn_agent_boot.trn_boot import _ntff_profile_via_ctypes
        mod = types.ModuleType("antenv.axon_hooks")
        _h = {}
        mod.set_axon_ntff_profile_hook = lambda h: _h.__setitem__("h", h)
        mod.get_axon_ntff_profile_hook = lambda: _h.get("h")
        sys.modules["antenv.axon_hooks"] = mod
        antenv.axon_hooks = mod
        mod.set_axon_ntff_profile_hook(
            _ntff_profile_via_ctypes("/opt/axon/libaxon_pjrt.so"))
        return True
    except Exception:
        return False





def build_program(INTERLEAVE=4, gps=(), redsplit=False, wbufs=2,
                  stagger=False, psf=2, pst=2, pss=2, dbufs=None, fold2=True, fold3=False, dup=True, qdve=False, g2dve=False, cpdve=False, smaj=False, cpdma=False, nodd=False, pairup=True, upg=1, d0dve=True):
    from contextlib import ExitStack
    import concourse.bacc as bacc
    import concourse.mybir as mybir
    import concourse.tile as tile

    gps = frozenset(gps)
    FP = mybir.dt.float32
    BF = mybir.dt.bfloat16
    I32 = mybir.dt.int32
    AF = mybir.ActivationFunctionType
    ALU = mybir.AluOpType
    AX = mybir.AxisListType

    OPS = _register_custom_ops()
    RSQ_F, RSQ_NR, GN2_F = OPS["ANT_RSQ_F"], OPS["ANT_RSQ_NR"], OPS["ANT_GN2_F"]

    nc = bacc.Bacc()
    # x pre-transposed on host: [block, half, i, j, d]
    x_d = nc.dram_tensor("xp", (NBLK, 2, C_IN, R, D_DIM), BF,
                         kind="ExternalInput")
    # x0 rows: [block, half, j, d]
    x0_d = nc.dram_tensor("x0p", (NBLK, 2, R, D_DIM), BF, kind="ExternalInput")
    w_d = nc.dram_tensor("w_rep", (128, W), BF, kind="ExternalInput")
    id_d = nc.dram_tensor("ident2", (128, 64), BF, kind="ExternalInput")
    # output in SBUF-natural order: [block, half, o, j, d]
    out_d = nc.dram_tensor("out_p", (NBLK, 2, C_OUT, R, D_DIM), BF,
                           kind="ExternalOutput")

    ctx = ExitStack()
    with ctx:
        tc = ctx.enter_context(tile.TileContext(nc))
        const = ctx.enter_context(tc.tile_pool(name="const", bufs=1))
        xg_p = ctx.enter_context(tc.tile_pool(name="xg", bufs=NBLK))
        work = ctx.enter_context(tc.tile_pool(name="work", bufs=wbufs))
        deep = ctx.enter_context(tc.tile_pool(name="deep", bufs=dbufs or max(2, INTERLEAVE)))
        ab_p = ctx.enter_context(tc.tile_pool(name="ab", bufs=NBLK))
        red_p = ctx.enter_context(tc.tile_pool(name="red", bufs=max(2, INTERLEAVE)))
        ps_f = ctx.enter_context(tc.tile_pool(name="psf", bufs=psf, space="PSUM"))
        ps_t = ctx.enter_context(tc.tile_pool(name="pst", bufs=pst, space="PSUM"))
        ps_s = ctx.enter_context(tc.tile_pool(name="pss", bufs=pss, space="PSUM"))

        def eng(name):
            return nc.gpsimd if name in gps else nc.vector

        # ---- constants (all contiguous DMAs)
        w_g = const.tile([128, W], BF, tag="wg")
        nc.sync.dma_start(w_g[:, :], w_d[:, :])
        ident = const.tile([128, 64], BF, tag="ident")
        nc.sync.dma_start(ident[:, :], id_d[:, :])
        halfpi = const.tile([128, 1], FP, tag="halfpi")
        nc.vector.memset(halfpi[:, :], HALF_PI)

        def jbh(t, h, j):
            return t[64 * h:64 * h + 64, 64 * j:64 * j + 64]

        def b3(t):
            return t[:, :].rearrange("p (j d) -> p j d", d=64)

        def bcR(small_ap):      # [128, R] ap -> broadcast (p, j, 64)
            return small_ap.rearrange("p (j o) -> p j o", o=1) \
                .broadcast_to([128, R, 64])

        def emit_load(st):
            b = st["b"]
            X = xg_p.tile([128, W], BF, tag="xg")
            for h in (0, 1):
                nc.sync.dma_start(b3(X[64 * h:64 * h + 64, :]), x_d[b, h])
            A0 = ab_p.tile([128, W], BF, tag="a0")
            for h in (0, 1):
                nc.sync.dma_start(
                    A0[64 * h:64 * h + 64, :].rearrange("p (j d) -> p j d", d=64),
                    x0_d[b:b + 1, h].rearrange("b j d -> b j d")
                    .broadcast_to([64, R, 64]))
            st["X"], st["A"] = X, A0
            if not d0dve:
                emit_xt(st)

        def emit_xt(st):
            X = st["X"]
            tp = ps_t.tile([128, W], BF, tag="tp")
            for h in (0, 1):
                for r in range(R):
                    nc.tensor.transpose(jbh(tp, h, r), jbh(X, h, r),
                                        ident[64 * h:64 * h + 64, :])
            XT = xg_p.tile([128, W], BF, tag="xt")
            if cpdma:
                nc.sync.dma_start(XT[:, :], tp[:, :])
            else:
                (nc.vector.tensor_copy if cpdve else nc.scalar.copy)(
                    XT[:, :], tp[:, :])
            st["XT"] = XT

        def quake(pool, src_ap, shape, tagp, out_dt, nr=False):
            seed = pool.tile(shape, FP, tag=tagp + "sd")
            nc.vector.tensor_scalar(seed[:, :].bitcast(I32),
                                    src_ap.bitcast(I32), 1, -1,
                                    ALU.logical_shift_right, ALU.bitwise_xor)
            rr = pool.tile(shape, out_dt, tag=tagp + "rr")
            nc.vector._custom_dve(RSQ_F, out=rr[:, :], in0=src_ap,
                                  in1=seed[:, :], s0=RSQ_C1, s1=RSQ_C2)
            if not nr:
                return rr
            rr2 = pool.tile(shape, out_dt, tag=tagp + "r2")
            nc.vector._custom_dve(RSQ_NR, out=rr2[:, :], in0=src_ap,
                                  in1=rr[:, :], s0=1.5, s1=0.5)
            return rr2

        # ---------- iteration 0: per-block D0 + small f-chain
        def emit_d0(st):
            shape = [128, R]
            if d0dve:
                # D0 via DVE: no XT dependency -> starts right after the DMAs
                X, A0 = st["X"], st["A"]
                prod0 = deep.tile([128, W], BF, tag="xf")
                nc.vector.tensor_tensor(prod0[:, :], X[:, :], A0[:, :],
                                        ALU.mult)
                D0 = red_p.tile(shape, FP, tag="f0d")
                nc.vector.tensor_reduce(D0[:, :], b3(prod0), AX.X, ALU.add)
                D0ap = D0[:, :]
                q0 = red_p.tile(shape, FP, tag="f0q")
                nc.vector.tensor_tensor(q0[:, :], D0ap, D0ap, ALU.mult)
            else:
                XT = st["XT"]
                psD0 = ps_s.tile([128, R], FP, tag="d0")
                for h in (0, 1):
                    for r in range(R):
                        nc.tensor.matmul(
                            psD0[64 * h:64 * h + 64, r:r + 1],
                            jbh(XT, h, r),
                            XT[64 * h:64 * h + 64, 64 * r:64 * r + 1])
                D0ap = psD0[:, :]
                q0 = red_p.tile(shape, FP, tag="f0q")
                nc.scalar.activation(q0[:, :], D0ap, AF.Square)
            u0 = red_p.tile(shape, FP, tag="f0u")
            nc.vector.tensor_scalar(u0[:, :], q0[:, :], -1.0, 1.0 + EPS_U,
                                    ALU.mult, ALU.add)
            rr0 = quake(red_p, u0[:, :], shape, "f0", FP)
            zs0 = red_p.tile(shape, FP, tag="f0z")
            nc.vector.tensor_tensor(zs0[:, :], D0ap, rr0[:, :], ALU.mult)
            th0 = red_p.tile(shape, FP, tag="f0t")
            nc.scalar.activation(th0[:, :], zs0[:, :], AF.Arctan, scale=-1.0)
            f0 = red_p.tile(shape, BF, tag="f0v")
            nc.vector.scalar_tensor_tensor(f0[:, :], th0[:, :], HALF_PI,
                                           rr0[:, :], ALU.add, ALU.mult)
            st["f0"] = f0[:, :]

        def emit_factor(st, it):
            X, XT = st["X"], st["XT"]
            if it == 0:
                Xf = deep.tile([128, W], BF, tag="xf")
                nc.vector.tensor_tensor(b3(Xf), b3(X), bcR(st["f0"]), ALU.mult)
                st["Xf"] = Xf
                return
            AT = st["AT"]
            psD = ps_f.tile([128, W], FP, tag="mmf")
            for h in (0, 1):
                for r in range(R):
                    nc.tensor.matmul(jbh(psD, h, r), jbh(XT, h, r),
                                     jbh(AT, h, r))
            q = work.tile([128, W], FP, tag="ffq")
            if not qdve:
                nc.scalar.activation(q[:, :], psD[:, :], AF.Square)
            if nodd:
                Dd = psD
            else:
                Dd = work.tile([128, W], BF, tag="dd")
                nc.scalar.copy(Dd[:, :], psD[:, :])
            if qdve:
                nc.vector.tensor_tensor(q[:, :], Dd[:, :], Dd[:, :], ALU.mult)
            u = work.tile([128, W], FP, tag="ffu")
            eng("u").tensor_scalar(u[:, :], q[:, :], -1.0, 1.0 + EPS_U,
                                   ALU.mult, ALU.add)
            rr = quake(work, u[:, :], [128, W], "ff", BF)
            zs = work.tile([128, W], BF, tag="zs")
            nc.vector.tensor_tensor(zs[:, :], Dd[:, :], rr[:, :], ALU.mult)
            th = work.tile([128, W], BF, tag="th")
            nc.scalar.activation(th[:, :], zs[:, :], AF.Arctan, scale=-1.0)
            thp = work.tile([128, W], BF, tag="thp")
            nc.vector.tensor_scalar(thp[:, :], th[:, :], HALF_PI, None,
                                    ALU.add)
            f = work.tile([128, W], BF, tag="fv")
            nc.vector.tensor_tensor(f[:, :], thp[:, :], rr[:, :], ALU.mult)
            S = deep.tile([128, W], BF, tag="sg")
            nc.vector.tensor_tensor(S[:, :], w_g[:, :], f[:, :], ALU.mult)
            st["S"] = S


        def emit_factor_smaj(prs, it):
            # per-block matmuls + ACT copies first
            tiles = []
            for st in prs:
                X, XT, AT = st["X"], st["XT"], st["AT"]
                psD = ps_f.tile([128, W], FP, tag="mmf")
                for h in (0, 1):
                    for r in range(R):
                        nc.tensor.matmul(jbh(psD, h, r), jbh(XT, h, r),
                                         jbh(AT, h, r))
                q = work.tile([128, W], FP, tag="ffq")
                nc.scalar.activation(q[:, :], psD[:, :], AF.Square)
                Dd = work.tile([128, W], BF, tag="dd")
                nc.scalar.copy(Dd[:, :], psD[:, :])
                tiles.append((q, Dd))
            # DVE stages interleaved across the pair
            us = []
            for q, Dd in tiles:
                u = work.tile([128, W], FP, tag="ffu")
                nc.vector.tensor_scalar(u[:, :], q[:, :], -1.0, 1.0 + EPS_U,
                                        ALU.mult, ALU.add)
                us.append(u)
            seeds = []
            for u in us:
                seed = work.tile([128, W], FP, tag="ffsd")
                nc.vector.tensor_scalar(seed[:, :].bitcast(I32),
                                        u[:, :].bitcast(I32), 1, -1,
                                        ALU.logical_shift_right,
                                        ALU.bitwise_xor)
                seeds.append(seed)
            rrs = []
            for u, seed in zip(us, seeds):
                rr = work.tile([128, W], BF, tag="ffrr")
                nc.vector._custom_dve(RSQ_F, out=rr[:, :], in0=u[:, :],
                                      in1=seed[:, :], s0=RSQ_C1, s1=RSQ_C2)
                rrs.append(rr)
            zss = []
            for (q, Dd), rr in zip(tiles, rrs):
                zs = work.tile([128, W], BF, tag="zs")
                nc.vector.tensor_tensor(zs[:, :], Dd[:, :], rr[:, :], ALU.mult)
                zss.append(zs)
            ths = []
            for zs in zss:
                th = work.tile([128, W], BF, tag="th")
                nc.scalar.activation(th[:, :], zs[:, :], AF.Arctan, scale=-1.0)
                ths.append(th)
            thps = []
            for th in ths:
                thp = work.tile([128, W], BF, tag="zs")
                nc.vector.tensor_scalar(thp[:, :], th[:, :], HALF_PI, None,
                                        ALU.add)
                thps.append(thp)
            fs = []
            for thp, rr in zip(thps, rrs):
                f = work.tile([128, W], BF, tag="th")
                nc.vector.tensor_tensor(f[:, :], thp[:, :], rr[:, :], ALU.mult)
                fs.append(f)
            for st, f in zip(prs, fs):
                S = deep.tile([128, W], BF, tag="sg")
                nc.vector.tensor_tensor(S[:, :], w_g[:, :], f[:, :], ALU.mult)
                st["S"] = S
        def emit_gmm(st, it):
            X = st["X"]
            psG = ps_f.tile([128, W], FP, tag="mmf")
            if it == 0:
                Xf = st["Xf"]
                for h in (0, 1):
                    for c in (0, 512):
                        nc.tensor.matmul(psG[64 * h:64 * h + 64, c:c + 512],
                                         w_g[64 * h:64 * h + 64, 0:64],
                                         Xf[64 * h:64 * h + 64, c:c + 512])
            else:
                S = st["S"]
                for h in (0, 1):
                    for r in range(R):
                        nc.tensor.matmul(jbh(psG, h, r), jbh(S, h, r),
                                         jbh(X, h, r))
            Gd = deep.tile([128, W], BF, tag="gd")
            nc.scalar.copy(Gd[:, :], psG[:, :])
            # write this block's prod/g2 slices now so psG frees in ACT order
            pg, idx = st["pg"], st["pgidx"]
            off = 2 * W * idx
            if g2dve:
                nc.vector.tensor_tensor(pg[:, off + W:off + 2 * W],
                                        Gd[:, :], Gd[:, :], ALU.mult)
            else:
                nc.scalar.activation(pg[:, off + W:off + 2 * W],
                                     psG[:, :], AF.Square)
            eng("prod").tensor_tensor(pg[:, off:off + W], st["A"][:, :],
                                      Gd[:, :], ALU.mult)
            st["Gd"] = Gd

        def emit_update(sts, it):
            npair = len(sts)
            pg = sts[0]["pg"]
            nred = 2 * R * npair
            red = red_p.tile([128, nred], FP, tag="red")
            # stage 1: pairwise fold at bf16 2x mode (halves reduce volume)
            fold = work.tile([128, W * npair], BF, tag="fold")
            pv = pg[:, :].rearrange("p (s two q) -> p s two q", two=2, q=32)
            nc.vector.tensor_tensor(
                fold[:, :].rearrange("p (s q) -> p s q", q=32),
                pv[:, :, 0, :], pv[:, :, 1, :], ALU.add)
            if fold2:
                fb = work.tile([128, W * npair // 2], BF, tag="fold2")
                fv2 = fold[:, :].rearrange("p (s two q) -> p s two q",
                                           two=2, q=16)
                nc.vector.tensor_tensor(
                    fb[:, :].rearrange("p (s q) -> p s q", q=16),
                    fv2[:, :, 0, :], fv2[:, :, 1, :], ALU.add)
                if fold3:
                    fc = work.tile([128, W * npair // 4], BF, tag="fold3")
                    fv3 = fb[:, :].rearrange("p (s two q) -> p s two q",
                                             two=2, q=8)
                    nc.vector.tensor_tensor(
                        fc[:, :].rearrange("p (s q) -> p s q", q=8),
                        fv3[:, :, 0, :], fv3[:, :, 1, :], ALU.add)
                    fb = fc
                    fview, qq = fb[:, :].rearrange("p (s q) -> p s q", q=8), 8
                else:
                    fview, qq = fb[:, :].rearrange("p (s q) -> p s q", q=16), 16
            else:
                fview, qq = fold[:, :].rearrange("p (s q) -> p s q", q=32), 32
            for c0 in range(0, npair, 2):
                hi = min(c0 + 2, npair)
                seg = slice(2 * R * c0, 2 * R * hi)
                nc.vector.tensor_reduce(
                    red[:, seg].rearrange("p (s j) -> p s j", j=R),
                    fview[:, 2 * R * c0:2 * R * hi], AX.X, ALU.add)
            # red cols: [idx][kind][j]: coefA at kind 0, gnr at kind 1
            rv = red[:, :].rearrange("p (i k j) -> p i k j", k=2, j=R)
            shape = [128, R * npair]
            coefA = red[:, :].rearrange("p (i k j) -> p (i k) j", k=2, j=R)
            # strided views
            cview = rv[:, :, 0, :]          # [128, npair, R]
            gview = rv[:, :, 1, :]
            gn2 = red_p.tile(shape, FP, tag="gn2")
            g3 = gn2[:, :].rearrange("p (i j) -> p i j", j=R)
            nc.vector._custom_dve(GN2_F, out=g3, in0=gview, in1=cview,
                                  s0=1e-30)
            rg = quake(red_p, gn2[:, :], shape, "rg", FP, nr=True)
            gn = red_p.tile(shape, FP, tag="gn")
            nc.vector.tensor_tensor(gn[:, :], gn2[:, :], rg[:, :], ALU.mult)
            cosg = red_p.tile(shape, FP, tag="cosg")
            nc.scalar.activation(cosg[:, :], gn[:, :], AF.Sin,
                                 bias=halfpi[:, 0:1])
            s1t = red_p.tile(shape, FP, tag="s1t")
            nc.scalar.activation(s1t[:, :], gn[:, :], AF.Sin)
            sc = red_p.tile(shape, FP, tag="sc")
            nc.vector.tensor_tensor(sc[:, :], s1t[:, :], rg[:, :], ALU.mult)
            t9 = red_p.tile(shape, FP, tag="t9")
            nc.vector.scalar_tensor_tensor(
                t9[:, :].rearrange("p (i j) -> p i j", j=R), sc[:, :]
                .rearrange("p (i j) -> p i j", j=R), -1.0, cview,
                ALU.mult, ALU.mult)
            alpha = red_p.tile(shape, BF, tag="alpha")
            nc.vector.tensor_tensor(alpha[:, :], cosg[:, :], t9[:, :], ALU.add)
            scb = red_p.tile(shape, BF, tag="scb")
            nc.vector.tensor_copy(scb[:, :], sc[:, :])
            if dup:
                a2 = red_p.tile([128, 2 * R * npair], BF, tag="a2")
                nc.vector.tensor_copy(
                    a2[:, :].rearrange("p (j two) -> p j two", two=2),
                    alpha[:, :].rearrange("p (j o) -> p j o", o=1)
                    .broadcast_to([128, R * npair, 2]))
                s2 = red_p.tile([128, 2 * R * npair], BF, tag="s2")
                nc.vector.tensor_copy(
                    s2[:, :].rearrange("p (j two) -> p j two", two=2),
                    scb[:, :].rearrange("p (j o) -> p j o", o=1)
                    .broadcast_to([128, R * npair, 2]))
            last = it == 2
            for idx, st in enumerate(sts):
                A, Gd = st["A"], st["Gd"]
                t1 = work.tile([128, W], BF, tag="scr1")
                t2 = work.tile([128, W], BF, tag="scr2")
                if dup:
                    def v4(t):
                        return t[:, :].rearrange(
                            "p (j o two) -> p j o two", two=2, o=32)
                    def bc4(small, i0):
                        return small[:, 2 * R * i0:2 * R * i0 + 2 * R] \
                            .rearrange("p (j o two) -> p j o two", o=1, two=2) \
                            .broadcast_to([128, R, 32, 2])
                    nc.vector.tensor_tensor(v4(t1), v4(A), bc4(a2, idx),
                                            ALU.mult)
                    nc.vector.tensor_tensor(v4(t2), v4(Gd), bc4(s2, idx),
                                            ALU.mult)
                else:
                    eng("t1").tensor_tensor(b3(t1), b3(A),
                                            bcR(alpha[:, R * idx:R * idx + R]),
                                            ALU.mult)
                    eng("t2").tensor_tensor(b3(t2), b3(Gd),
                                            bcR(scb[:, R * idx:R * idx + R]),
                                            ALU.mult)
                An = ab_p.tile([128, W], BF, tag="agf" if last else "ag")
                nc.vector.tensor_tensor(An[:, :], t1[:, :], t2[:, :], ALU.add)
                st["A"] = An
                if not last:
                    tp = ps_t.tile([128, W], BF, tag="tp")
                    for h in (0, 1):
                        for r in range(R):
                            nc.tensor.transpose(jbh(tp, h, r), jbh(An, h, r),
                                                ident[64 * h:64 * h + 64, :])
                    AT = ab_p.tile([128, W], BF, tag="at")
                    if cpdma:
                        nc.sync.dma_start(AT[:, :], tp[:, :])
                    else:
                        (nc.vector.tensor_copy if cpdve else nc.scalar.copy)(
                            AT[:, :], tp[:, :])
                    st["AT"] = AT
                else:
                    for h in (0, 1):
                        nc.sync.dma_start(
                            out_d[st["b"], h],
                            b3(An[64 * h:64 * h + 64, :]))

        all_sts = [{"b": b} for b in range(NBLK)]
        for st in all_sts:
            emit_load(st)
            emit_d0(st)
        if d0dve:
            for st in all_sts:
                emit_xt(st)
        for b0 in range(0, NBLK, INTERLEAVE):
            sts = all_sts[b0:b0 + INTERLEAVE]
            for it in range(3):
                if pairup:
                    for st in sts:
                        emit_factor(st, it)
                    for i0 in range(0, len(sts), upg):
                        pr = sts[i0:i0 + upg]
                        pgp = work.tile([128, 2 * W * len(pr)], BF, tag="pg")
                        for idx, st in enumerate(pr):
                            st["pg"], st["pgidx"] = pgp, idx
                            emit_gmm(st, it)
                        emit_update(pr, it)
                    continue
                pg = work.tile([128, 2 * W * len(sts)], BF, tag="pg")
                for idx, st in enumerate(sts):
                    st["pg"], st["pgidx"] = pg, idx
                if smaj and it > 0:
                    for i0 in range(0, len(sts), 2):
                        emit_factor_smaj(sts[i0:i0 + 2], it)
                    for st in sts:
                        emit_gmm(st, it)
                elif stagger:
                    n = len(sts)
                    for k in range(n + 1):
                        if k < n:
                            emit_factor(sts[k], it)
                        if k > 0:
                            emit_gmm(sts[k - 1], it)
                else:
                    for st in sts:
                        emit_factor(st, it)
                    for st in sts:
                        emit_gmm(st, it)
                emit_update(sts, it)
    nc.compile()
    return nc


def _get_program(**kw):
    key = tuple(sorted((k, tuple(v) if isinstance(v, (list, tuple, set, frozenset))
                        else v) for k, v in kw.items()))
    if key not in _COMPILED:
        _COMPILED[key] = build_program(**kw)
    return _COMPILED[key]


def kernel(x, w_raw, _trace=False, **bkw):
    import ml_dtypes
    from concourse.bass_utils import run_bass_kernel_spmd
    if _trace:
        _trace = _ensure_trace_hook()

    bf16 = ml_dtypes.bfloat16
    x = np.asarray(x, f32)
    w_raw = np.asarray(w_raw, f32)
    B, L, C_in, d = x.shape
    N = B * L
    w = np.exp((w_raw - f32(np.log(C_in))).astype(f32)).astype(f32)
    w = (w / w.sum(axis=0, keepdims=True)).astype(f32)

    xr = x.reshape(N, C_in, d)
    # per core: [NBLK, 2, R, i, d] -> transpose to [NBLK, 2, i, R, d]
    xcore = xr.reshape(N_CORES, NBLK, 2, R, C_in, d)
    xp = np.ascontiguousarray(xcore.transpose(0, 1, 2, 4, 3, 5)).astype(bf16)
    x0p = np.ascontiguousarray(xcore[:, :, :, :, 0, :]).astype(bf16)
    w_rep = np.ascontiguousarray(
        np.broadcast_to(w.T.reshape(1, 64, 1, 64), (2, 64, R, 64))
        .transpose(0, 3, 2, 1).reshape(128, W)).astype(bf16)
    # w_rep[p, (j, o)]: lower/upper halves identical, = w[i=p%64, o]
    w_rep = np.ascontiguousarray(
        np.tile(np.repeat(w[None, :, :], 1, axis=0), (2, 1, 1))  # (2,64,64)
        .reshape(2, 64, 1, 64).repeat(R, axis=2).reshape(2 * 64, R * 64)
        ).astype(bf16)
    ident2 = np.tile(np.eye(64, dtype=bf16), (2, 1))

    nc = _get_program(**bkw)
    in_maps = []
    for k in range(N_CORES):
        in_maps.append({
            "xp": xp[k],
            "x0p": x0p[k],
            "w_rep": w_rep,
            "ident2": ident2,
        })
    res = run_bass_kernel_spmd(nc, in_maps, core_ids=list(range(N_CORES)),
                               trace=_trace)
    # out_p: [NBLK, 2, o, j, d] per core -> rows
    outs = []
    for k in range(N_CORES):
        op = res.results[k]["out_p"]          # (NBLK, 2, 64, R, 64)
        outs.append(np.ascontiguousarray(op.transpose(0, 1, 3, 2, 4))
                    .reshape(ROWS_PER_CORE, C_OUT, d))
    out = np.concatenate(outs, axis=0)
    if _trace:
        kernel.last_exec_time_ns = res.exec_time_ns
        kernel.last_results = res
    return out.reshape(B, L, C_OUT, d).astype(f32)


# revision 13
# speedup vs baseline: 1.1491x; 1.0058x over previous
"""Trainium2 Bass kernel v3 for nn_MfdFC. See kernel_v2 docstring for math.

v3 over v2:
- host pre-transposes x into per-(block,half) [i, j, d] layout and w into the
  replicated [128, 1024] SBUF image -> all input DMAs are contiguous; the
  output is written in SBUF-natural [o, j, d] order and re-transposed on host.
- the per-block [128,16] "smalls" pipelines (iter-0 f-chain, update-phase
  cos/sin/rsqrt chain) run once per interleave-PAIR on [128,32] tiles, and
  the coefA/|G|^2 reductions of a pair are fused into ONE 4096-wide reduce.
- optional GPSIMD offload for selected elementwise passes (t2, prod).
"""
import math
import numpy as np

f32 = np.float32

C_IN = 64
C_OUT = 64
D_DIM = 64
ROWS_PER_CORE = 128
N_CORES = 8
R = 16
NBLK = 4
W = 64 * R
RSQ_C1 = 1.7584694439735017e-30
RSQ_C2 = -2.755803843779718e-20
HALF_PI = float(f32(math.pi / 2.0))
EPS_U = float(f32(2.0 ** -22))

_COMPILED = {}

def _register_custom_ops():
    import concourse.dve_ops as dve_ops
    from concourse.dve_ops import DveOp
    from concourse.dve_spec import (
        Spec, Src0, Src1, C0, C1, lower, maxx, _has_src1 as has_src1,
    )
    from concourse.dve_uop import DveOpSpec
    from concourse.dve_table_gen import dve_ver_for

    if "ANT_RSQ_F" in dve_ops._SUB_OPCODE_FOR_NAME:
        return {n: op for n, op in ((o.name, o) for o in dve_ops.OPS)
                if n.startswith("ANT_")}

    def _ref_rsq_f(in0, in1, s0, s1, imm2):
        u = np.asarray(in0, f32)
        nt = np.asarray(in1, f32)
        m1 = (nt * f32(s0)).astype(f32)
        m2 = (m1 * nt).astype(f32)
        m3 = (m2 * f32(s0)).astype(f32)
        t = (m3 * u).astype(f32)
        return ((t + f32(s1)) * nt).astype(f32)

    _m1 = Src1 * C0
    _m3 = (_m1 * Src1) * C0
    RSQ_F = DveOp("ANT_RSQ_F",
                  Spec(body=((_m3 * Src0) + C1) * Src1, reference=_ref_rsq_f),
                  subdim=False, uops_sha={})

    def _ref_rsq_nr(in0, in1, s0, s1, imm2):
        u = np.asarray(in0, f32); y = np.asarray(in1, f32)
        a = (u * y).astype(f32)
        b = (a * y).astype(f32)
        return ((f32(s0) - (b * f32(s1)).astype(f32)) * y).astype(f32)

    RSQ_NR = DveOp("ANT_RSQ_NR",
                   Spec(body=(C0 - ((Src0 * Src1) * Src1) * C1) * Src1,
                        reference=_ref_rsq_nr),
                   subdim=False, uops_sha={})

    def _ref_gn2(in0, in1, s0, s1, imm2):
        raw = np.asarray(in0, f32); c = np.asarray(in1, f32)
        return np.maximum((raw - (c * c).astype(f32)).astype(f32), f32(s0))

    GN2_F = DveOp("ANT_GN2_F",
                  Spec(body=maxx(Src0 - Src1 * Src1, C0), reference=_ref_gn2),
                  subdim=False, uops_sha={})

    ops = [RSQ_F, RSQ_NR, GN2_F]
    base = dve_ops._CUSTOM_DVE_ROW_BASE + len(dve_ops.OPS)
    for i, op in enumerate(ops):
        dve_ops._SUB_OPCODE_FOR_NAME[op.name] = base + i
    for trn in ("TRN2",):
        ver = dve_ver_for(trn)
        for op in ops:
            uops = lower(op.spec, ver=ver)
            s = DveOpSpec(name=op.name, opcode=dve_ops.get_dve_sub_opcode(op.name),
                          uops=uops, rd1_en=has_src1(op.spec))
            op.uops_sha[ver] = s.sha(ver)
    dve_ops.OPS.extend(ops)
    dve_ops.CUSTOM_DVE_SPECS.update({op.name: op.spec for op in ops})
    return {op.name: op for op in ops}



def _ensure_trace_hook():
    try:
        from antenv.axon_hooks import get_axon_ntff_profile_hook
        return get_axon_ntff_profile_hook() is not None
    except ImportError:
        pass
    try:
        import sys, types
        import antenv
        from trn_agent_boot.trn_boot import _ntff_profile_via_ctypes
        mod = types.ModuleType("antenv.axon_hooks")
        _h = {}
        mod.set_axon_ntff_profile_hook = lambda h: _h.__setitem__("h", h)
        mod.get_axon_ntff_profile_hook = lambda: _h.get("h")
        sys.modules["antenv.axon_hooks"] = mod
        antenv.axon_hooks = mod
        mod.set_axon_ntff_profile_hook(
            _ntff_profile_via_ctypes("/opt/axon/libaxon_pjrt.so"))
        return True
    except Exception:
        return False





def build_program(INTERLEAVE=4, gps=(), redsplit=False, wbufs=3,
                  stagger=False, psf=2, pst=2, pss=2, dbufs=None, fold2=True, fold3=False, dup=True, qdve=False, g2dve=False, cpdve=False, smaj=False, cpdma=False, nodd=False, pairup=True, upg=1, d0dve=True, f0dup=False):
    from contextlib import ExitStack
    import concourse.bacc as bacc
    import concourse.mybir as mybir
    import concourse.tile as tile

    gps = frozenset(gps)
    FP = mybir.dt.float32
    BF = mybir.dt.bfloat16
    I32 = mybir.dt.int32
    AF = mybir.ActivationFunctionType
    ALU = mybir.AluOpType
    AX = mybir.AxisListType

    OPS = _register_custom_ops()
    RSQ_F, RSQ_NR, GN2_F = OPS["ANT_RSQ_F"], OPS["ANT_RSQ_NR"], OPS["ANT_GN2_F"]

    nc = bacc.Bacc()
    # x pre-transposed on host: [block, half, i, j, d]
    x_d = nc.dram_tensor("xp", (NBLK, 2, C_IN, R, D_DIM), BF,
                         kind="ExternalInput")
    # x0 rows: [block, half, j, d]
    x0_d = nc.dram_tensor("x0p", (NBLK, 2, R, D_DIM), BF, kind="ExternalInput")
    w_d = nc.dram_tensor("w_rep", (128, W), BF, kind="ExternalInput")
    id_d = nc.dram_tensor("ident2", (128, 64), BF, kind="ExternalInput")
    # output in SBUF-natural order: [block, half, o, j, d]
    out_d = nc.dram_tensor("out_p", (NBLK, 2, C_OUT, R, D_DIM), BF,
                           kind="ExternalOutput")

    ctx = ExitStack()
    with ctx:
        tc = ctx.enter_context(tile.TileContext(nc))
        const = ctx.enter_context(tc.tile_pool(name="const", bufs=1))
        xg_p = ctx.enter_context(tc.tile_pool(name="xg", bufs=NBLK))
        work = ctx.enter_context(tc.tile_pool(name="work", bufs=wbufs))
        deep = ctx.enter_context(tc.tile_pool(name="deep", bufs=dbufs or max(2, INTERLEAVE)))
        ab_p = ctx.enter_context(tc.tile_pool(name="ab", bufs=NBLK))
        red_p = ctx.enter_context(tc.tile_pool(name="red", bufs=max(2, INTERLEAVE)))
        ps_f = ctx.enter_context(tc.tile_pool(name="psf", bufs=psf, space="PSUM"))
        ps_t = ctx.enter_context(tc.tile_pool(name="pst", bufs=pst, space="PSUM"))
        ps_s = ctx.enter_context(tc.tile_pool(name="pss", bufs=pss, space="PSUM"))

        def eng(name):
            return nc.gpsimd if name in gps else nc.vector

        # ---- constants (DMAs deferred until after block-0 loads)
        w_g = const.tile([128, W], BF, tag="wg")
        ident = const.tile([128, 64], BF, tag="ident")
        halfpi = const.tile([128, 1], FP, tag="halfpi")
        nc.vector.memset(halfpi[:, :], HALF_PI)

        def emit_const_dmas():
            nc.sync.dma_start(w_g[:, :], w_d[:, :])
            nc.sync.dma_start(ident[:, :], id_d[:, :])

        def jbh(t, h, j):
            return t[64 * h:64 * h + 64, 64 * j:64 * j + 64]

        def b3(t):
            return t[:, :].rearrange("p (j d) -> p j d", d=64)

        def bcR(small_ap):      # [128, R] ap -> broadcast (p, j, 64)
            return small_ap.rearrange("p (j o) -> p j o", o=1) \
                .broadcast_to([128, R, 64])

        def emit_load(st):
            b = st["b"]
            X = xg_p.tile([128, W], BF, tag="xg")
            for h in (0, 1):
                nc.sync.dma_start(b3(X[64 * h:64 * h + 64, :]), x_d[b, h])
            A0 = ab_p.tile([128, W], BF, tag="a0")
            for h in (0, 1):
                nc.sync.dma_start(
                    A0[64 * h:64 * h + 64, :].rearrange("p (j d) -> p j d", d=64),
                    x0_d[b:b + 1, h].rearrange("b j d -> b j d")
                    .broadcast_to([64, R, 64]))
            st["X"], st["A"] = X, A0
            if not d0dve:
                emit_xt(st)

        def emit_xt(st):
            X = st["X"]
            tp = ps_t.tile([128, W], BF, tag="tp")
            for h in (0, 1):
                for r in range(R):
                    nc.tensor.transpose(jbh(tp, h, r), jbh(X, h, r),
                                        ident[64 * h:64 * h + 64, :])
            XT = xg_p.tile([128, W], BF, tag="xt")
            if cpdma:
                nc.sync.dma_start(XT[:, :], tp[:, :])
            else:
                (nc.vector.tensor_copy if cpdve else nc.scalar.copy)(
                    XT[:, :], tp[:, :])
            st["XT"] = XT

        def quake(pool, src_ap, shape, tagp, out_dt, nr=False):
            seed = pool.tile(shape, FP, tag=tagp + "sd")
            nc.vector.tensor_scalar(seed[:, :].bitcast(I32),
                                    src_ap.bitcast(I32), 1, -1,
                                    ALU.logical_shift_right, ALU.bitwise_xor)
            rr = pool.tile(shape, out_dt, tag=tagp + "rr")
            nc.vector._custom_dve(RSQ_F, out=rr[:, :], in0=src_ap,
                                  in1=seed[:, :], s0=RSQ_C1, s1=RSQ_C2)
            if not nr:
                return rr
            rr2 = pool.tile(shape, out_dt, tag=tagp + "r2")
            nc.vector._custom_dve(RSQ_NR, out=rr2[:, :], in0=src_ap,
                                  in1=rr[:, :], s0=1.5, s1=0.5)
            return rr2

        # ---------- iteration 0: per-block D0 + small f-chain
        def emit_d0(st):
            shape = [128, R]
            if d0dve:
                # D0 via DVE: no XT dependency -> starts right after the DMAs
                X, A0 = st["X"], st["A"]
                prod0 = deep.tile([128, W], BF, tag="xf")
                nc.vector.tensor_tensor(prod0[:, :], X[:, :], A0[:, :],
                                        ALU.mult)
                D0 = red_p.tile(shape, FP, tag="f0d")
                nc.vector.tensor_reduce(D0[:, :], b3(prod0), AX.X, ALU.add)
                D0ap = D0[:, :]
                q0 = red_p.tile(shape, FP, tag="f0q")
                nc.vector.tensor_tensor(q0[:, :], D0ap, D0ap, ALU.mult)
            else:
                XT = st["XT"]
                psD0 = ps_s.tile([128, R], FP, tag="d0")
                for h in (0, 1):
                    for r in range(R):
                        nc.tensor.matmul(
                            psD0[64 * h:64 * h + 64, r:r + 1],
                            jbh(XT, h, r),
                            XT[64 * h:64 * h + 64, 64 * r:64 * r + 1])
                D0ap = psD0[:, :]
                q0 = red_p.tile(shape, FP, tag="f0q")
                nc.scalar.activation(q0[:, :], D0ap, AF.Square)
            u0 = red_p.tile(shape, FP, tag="f0u")
            nc.vector.tensor_scalar(u0[:, :], q0[:, :], -1.0, 1.0 + EPS_U,
                                    ALU.mult, ALU.add)
            rr0 = quake(red_p, u0[:, :], shape, "f0", FP)
            zs0 = red_p.tile(shape, FP, tag="f0z")
            nc.vector.tensor_tensor(zs0[:, :], D0ap, rr0[:, :], ALU.mult)
            th0 = red_p.tile(shape, FP, tag="f0t")
            nc.scalar.activation(th0[:, :], zs0[:, :], AF.Arctan, scale=-1.0)
            f0 = red_p.tile(shape, BF, tag="f0v")
            nc.vector.scalar_tensor_tensor(f0[:, :], th0[:, :], HALF_PI,
                                           rr0[:, :], ALU.add, ALU.mult)
            st["f0"] = f0[:, :]
            if f0dup:
                f0d = red_p.tile([128, 2 * R], BF, tag="f0d2")
                nc.vector.tensor_copy(
                    f0d[:, :].rearrange("p (j two) -> p j two", two=2),
                    f0[:, :].rearrange("p (j o) -> p j o", o=1)
                    .broadcast_to([128, R, 2]))
                st["f0d"] = f0d

        def emit_factor(st, it):
            X, XT = st["X"], st["XT"]
            if it == 0:
                Xf = deep.tile([128, W], BF, tag="xf")
                if f0dup:
                    def v4(t):
                        return t[:, :].rearrange(
                            "p (j o two) -> p j o two", two=2, o=32)
                    f0b = st["f0d"][:, :] \
                        .rearrange("p (j o two) -> p j o two", o=1, two=2) \
                        .broadcast_to([128, R, 32, 2])
                    nc.vector.tensor_tensor(v4(Xf), v4(X), f0b, ALU.mult)
                else:
                    nc.vector.tensor_tensor(b3(Xf), b3(X), bcR(st["f0"]),
                                            ALU.mult)
                st["Xf"] = Xf
                return
            AT = st["AT"]
            psD = ps_f.tile([128, W], FP, tag="mmf")
            for h in (0, 1):
                for r in range(R):
                    nc.tensor.matmul(jbh(psD, h, r), jbh(XT, h, r),
                                     jbh(AT, h, r))
            q = work.tile([128, W], FP, tag="ffq")
            if not qdve:
                nc.scalar.activation(q[:, :], psD[:, :], AF.Square)
            if nodd:
                Dd = psD
            else:
                Dd = work.tile([128, W], BF, tag="dd")
                nc.scalar.copy(Dd[:, :], psD[:, :])
            if qdve:
                nc.vector.tensor_tensor(q[:, :], Dd[:, :], Dd[:, :], ALU.mult)
            u = work.tile([128, W], FP, tag="ffu")
            eng("u").tensor_scalar(u[:, :], q[:, :], -1.0, 1.0 + EPS_U,
                                   ALU.mult, ALU.add)
            rr = quake(work, u[:, :], [128, W], "ff", BF)
            zs = work.tile([128, W], BF, tag="zs")
            nc.vector.tensor_tensor(zs[:, :], Dd[:, :], rr[:, :], ALU.mult)
            th = work.tile([128, W], BF, tag="th")
            nc.scalar.activation(th[:, :], zs[:, :], AF.Arctan, scale=-1.0)
            thp = work.tile([128, W], BF, tag="thp")
            nc.vector.tensor_scalar(thp[:, :], th[:, :], HALF_PI, None,
                                    ALU.add)
            f = work.tile([128, W], BF, tag="fv")
            nc.vector.tensor_tensor(f[:, :], thp[:, :], rr[:, :], ALU.mult)
            S = deep.tile([128, W], BF, tag="sg")
            nc.vector.tensor_tensor(S[:, :], w_g[:, :], f[:, :], ALU.mult)
            st["S"] = S


        def emit_factor_smaj(prs, it):
            # per-block matmuls + ACT copies first
            tiles = []
            for st in prs:
                X, XT, AT = st["X"], st["XT"], st["AT"]
                psD = ps_f.tile([128, W], FP, tag="mmf")
                for h in (0, 1):
                    for r in range(R):
                        nc.tensor.matmul(jbh(psD, h, r), jbh(XT, h, r),
                                         jbh(AT, h, r))
                q = work.tile([128, W], FP, tag="ffq")
                nc.scalar.activation(q[:, :], psD[:, :], AF.Square)
                Dd = work.tile([128, W], BF, tag="dd")
                nc.scalar.copy(Dd[:, :], psD[:, :])
                tiles.append((q, Dd))
            # DVE stages interleaved across the pair
            us = []
            for q, Dd in tiles:
                u = work.tile([128, W], FP, tag="ffu")
                nc.vector.tensor_scalar(u[:, :], q[:, :], -1.0, 1.0 + EPS_U,
                                        ALU.mult, ALU.add)
                us.append(u)
            seeds = []
            for u in us:
                seed = work.tile([128, W], FP, tag="ffsd")
                nc.vector.tensor_scalar(seed[:, :].bitcast(I32),
                                        u[:, :].bitcast(I32), 1, -1,
                                        ALU.logical_shift_right,
                                        ALU.bitwise_xor)
                seeds.append(seed)
            rrs = []
            for u, seed in zip(us, seeds):
                rr = work.tile([128, W], BF, tag="ffrr")
                nc.vector._custom_dve(RSQ_F, out=rr[:, :], in0=u[:, :],
                                      in1=seed[:, :], s0=RSQ_C1, s1=RSQ_C2)
                rrs.append(rr)
            zss = []
            for (q, Dd), rr in zip(tiles, rrs):
                zs = work.tile([128, W], BF, tag="zs")
                nc.vector.tensor_tensor(zs[:, :], Dd[:, :], rr[:, :], ALU.mult)
                zss.append(zs)
            ths = []
            for zs in zss:
                th = work.tile([128, W], BF, tag="th")
                nc.scalar.activation(th[:, :], zs[:, :], AF.Arctan, scale=-1.0)
                ths.append(th)
            thps = []
            for th in ths:
                thp = work.tile([128, W], BF, tag="zs")
                nc.vector.tensor_scalar(thp[:, :], th[:, :], HALF_PI, None,
                                        ALU.add)
                thps.append(thp)
            fs = []
            for thp, rr in zip(thps, rrs):
                f = work.tile([128, W], BF, tag="th")
                nc.vector.tensor_tensor(f[:, :], thp[:, :], rr[:, :], ALU.mult)
                fs.append(f)
            for st, f in zip(prs, fs):
                S = deep.tile([128, W], BF, tag="sg")
                nc.vector.tensor_tensor(S[:, :], w_g[:, :], f[:, :], ALU.mult)
                st["S"] = S
        def emit_gmm(st, it):
            X = st["X"]
            psG = ps_f.tile([128, W], FP, tag="mmf")
            if it == 0:
                Xf = st["Xf"]
                for h in (0, 1):
                    for c in (0, 512):
                        nc.tensor.matmul(psG[64 * h:64 * h + 64, c:c + 512],
                                         w_g[64 * h:64 * h + 64, 0:64],
                                         Xf[64 * h:64 * h + 64, c:c + 512])
            else:
                S = st["S"]
                for h in (0, 1):
                    for r in range(R):
                        nc.tensor.matmul(jbh(psG, h, r), jbh(S, h, r),
                                         jbh(X, h, r))
            Gd = deep.tile([128, W], BF, tag="gd")
            nc.scalar.copy(Gd[:, :], psG[:, :])
            # write this block's prod/g2 slices now so psG frees in ACT order
            pg, idx = st["pg"], st["pgidx"]
            off = 2 * W * idx
            if g2dve:
                nc.vector.tensor_tensor(pg[:, off + W:off + 2 * W],
                                        Gd[:, :], Gd[:, :], ALU.mult)
            else:
                nc.scalar.activation(pg[:, off + W:off + 2 * W],
                                     psG[:, :], AF.Square)
            eng("prod").tensor_tensor(pg[:, off:off + W], st["A"][:, :],
                                      Gd[:, :], ALU.mult)
            st["Gd"] = Gd

        def emit_update(sts, it):
            npair = len(sts)
            pg = sts[0]["pg"]
            nred = 2 * R * npair
            red = red_p.tile([128, nred], FP, tag="red")
            # stage 1: pairwise fold at bf16 2x mode (halves reduce volume)
            fold = work.tile([128, W * npair], BF, tag="fold")
            pv = pg[:, :].rearrange("p (s two q) -> p s two q", two=2, q=32)
            nc.vector.tensor_tensor(
                fold[:, :].rearrange("p (s q) -> p s q", q=32),
                pv[:, :, 0, :], pv[:, :, 1, :], ALU.add)
            if fold2:
                fb = work.tile([128, W * npair // 2], BF, tag="fold2")
                fv2 = fold[:, :].rearrange("p (s two q) -> p s two q",
                                           two=2, q=16)
                nc.vector.tensor_tensor(
                    fb[:, :].rearrange("p (s q) -> p s q", q=16),
                    fv2[:, :, 0, :], fv2[:, :, 1, :], ALU.add)
                if fold3:
                    fc = work.tile([128, W * npair // 4], BF, tag="fold3")
                    fv3 = fb[:, :].rearrange("p (s two q) -> p s two q",
                                             two=2, q=8)
                    nc.vector.tensor_tensor(
                        fc[:, :].rearrange("p (s q) -> p s q", q=8),
                        fv3[:, :, 0, :], fv3[:, :, 1, :], ALU.add)
                    fb = fc
                    fview, qq = fb[:, :].rearrange("p (s q) -> p s q", q=8), 8
                else:
                    fview, qq = fb[:, :].rearrange("p (s q) -> p s q", q=16), 16
            else:
                fview, qq = fold[:, :].rearrange("p (s q) -> p s q", q=32), 32
            for c0 in range(0, npair, 2):
                hi = min(c0 + 2, npair)
                seg = slice(2 * R * c0, 2 * R * hi)
                nc.vector.tensor_reduce(
                    red[:, seg].rearrange("p (s j) -> p s j", j=R),
                    fview[:, 2 * R * c0:2 * R * hi], AX.X, ALU.add)
            # red cols: [idx][kind][j]: coefA at kind 0, gnr at kind 1
            rv = red[:, :].rearrange("p (i k j) -> p i k j", k=2, j=R)
            shape = [128, R * npair]
            coefA = red[:, :].rearrange("p (i k j) -> p (i k) j", k=2, j=R)
            # strided views
            cview = rv[:, :, 0, :]          # [128, npair, R]
            gview = rv[:, :, 1, :]
            gn2 = red_p.tile(shape, FP, tag="gn2")
            g3 = gn2[:, :].rearrange("p (i j) -> p i j", j=R)
            nc.vector._custom_dve(GN2_F, out=g3, in0=gview, in1=cview,
                                  s0=1e-30)
            rg = quake(red_p, gn2[:, :], shape, "rg", FP, nr=True)
            gn = red_p.tile(shape, FP, tag="gn")
            nc.vector.tensor_tensor(gn[:, :], gn2[:, :], rg[:, :], ALU.mult)
            cosg = red_p.tile(shape, FP, tag="cosg")
            nc.scalar.activation(cosg[:, :], gn[:, :], AF.Sin,
                                 bias=halfpi[:, 0:1])
            s1t = red_p.tile(shape, FP, tag="s1t")
            nc.scalar.activation(s1t[:, :], gn[:, :], AF.Sin)
            sc = red_p.tile(shape, FP, tag="sc")
            nc.vector.tensor_tensor(sc[:, :], s1t[:, :], rg[:, :], ALU.mult)
            t9 = red_p.tile(shape, FP, tag="t9")
            nc.vector.scalar_tensor_tensor(
                t9[:, :].rearrange("p (i j) -> p i j", j=R), sc[:, :]
                .rearrange("p (i j) -> p i j", j=R), -1.0, cview,
                ALU.mult, ALU.mult)
            alpha = red_p.tile(shape, BF, tag="alpha")
            nc.vector.tensor_tensor(alpha[:, :], cosg[:, :], t9[:, :], ALU.add)
            scb = red_p.tile(shape, BF, tag="scb")
            nc.vector.tensor_copy(scb[:, :], sc[:, :])
            if dup:
                a2 = red_p.tile([128, 2 * R * npair], BF, tag="a2")
                nc.vector.tensor_copy(
                    a2[:, :].rearrange("p (j two) -> p j two", two=2),
                    alpha[:, :].rearrange("p (j o) -> p j o", o=1)
                    .broadcast_to([128, R * npair, 2]))
                s2 = red_p.tile([128, 2 * R * npair], BF, tag="s2")
                nc.vector.tensor_copy(
                    s2[:, :].rearrange("p (j two) -> p j two", two=2),
                    scb[:, :].rearrange("p (j o) -> p j o", o=1)
                    .broadcast_to([128, R * npair, 2]))
            last = it == 2
            for idx, st in enumerate(sts):
                A, Gd = st["A"], st["Gd"]
                t1 = work.tile([128, W], BF, tag="scr1")
                t2 = work.tile([128, W], BF, tag="scr2")
                if dup:
                    def v4(t):
                        return t[:, :].rearrange(
                            "p (j o two) -> p j o two", two=2, o=32)
                    def bc4(small, i0):
                        return small[:, 2 * R * i0:2 * R * i0 + 2 * R] \
                            .rearrange("p (j o two) -> p j o two", o=1, two=2) \
                            .broadcast_to([128, R, 32, 2])
                    nc.vector.tensor_tensor(v4(t1), v4(A), bc4(a2, idx),
                                            ALU.mult)
                    nc.vector.tensor_tensor(v4(t2), v4(Gd), bc4(s2, idx),
                                            ALU.mult)
                else:
                    eng("t1").tensor_tensor(b3(t1), b3(A),
                                            bcR(alpha[:, R * idx:R * idx + R]),
                                            ALU.mult)
                    eng("t2").tensor_tensor(b3(t2), b3(Gd),
                                            bcR(scb[:, R * idx:R * idx + R]),
                                            ALU.mult)
                An = ab_p.tile([128, W], BF, tag="agf" if last else "ag")
                nc.vector.tensor_tensor(An[:, :], t1[:, :], t2[:, :], ALU.add)
                st["A"] = An
                if not last:
                    tp = ps_t.tile([128, W], BF, tag="tp")
                    for h in (0, 1):
                        for r in range(R):
                            nc.tensor.transpose(jbh(tp, h, r), jbh(An, h, r),
                                                ident[64 * h:64 * h + 64, :])
                    AT = ab_p.tile([128, W], BF, tag="at")
                    if cpdma:
                        nc.sync.dma_start(AT[:, :], tp[:, :])
                    else:
                        (nc.vector.tensor_copy if cpdve else nc.scalar.copy)(
                            AT[:, :], tp[:, :])
                    st["AT"] = AT
                else:
                    for h in (0, 1):
                        nc.sync.dma_start(
                            out_d[st["b"], h],
                            b3(An[64 * h:64 * h + 64, :]))

        all_sts = [{"b": b} for b in range(NBLK)]
        for k, st in enumerate(all_sts):
            emit_load(st)
            if k == 0:
                emit_const_dmas()
            emit_d0(st)
        if d0dve:
            for st in all_sts:
                emit_xt(st)
        for b0 in range(0, NBLK, INTERLEAVE):
            sts = all_sts[b0:b0 + INTERLEAVE]
            for it in range(3):
                if pairup:
                    for st in sts:
                        emit_factor(st, it)
                    for i0 in range(0, len(sts), upg):
                        pr = sts[i0:i0 + upg]
                        pgp = work.tile([128, 2 * W * len(pr)], BF, tag="pg")
                        for idx, st in enumerate(pr):
                            st["pg"], st["pgidx"] = pgp, idx
                            emit_gmm(st, it)
                        emit_update(pr, it)
                    continue
                pg = work.tile([128, 2 * W * len(sts)], BF, tag="pg")
                for idx, st in enumerate(sts):
                    st["pg"], st["pgidx"] = pg, idx
                if smaj and it > 0:
                    for i0 in range(0, len(sts), 2):
                        emit_factor_smaj(sts[i0:i0 + 2], it)
                    for st in sts:
                        emit_gmm(st, it)
                elif stagger:
                    n = len(sts)
                    for k in range(n + 1):
                        if k < n:
                            emit_factor(sts[k], it)
                        if k > 0:
                            emit_gmm(sts[k - 1], it)
                else:
                    for st in sts:
                        emit_factor(st, it)
                    for st in sts:
                        emit_gmm(st, it)
                emit_update(sts, it)
    nc.compile()
    return nc


def _get_program(**kw):
    key = tuple(sorted((k, tuple(v) if isinstance(v, (list, tuple, set, frozenset))
                        else v) for k, v in kw.items()))
    if key not in _COMPILED:
        _COMPILED[key] = build_program(**kw)
    return _COMPILED[key]


def kernel(x, w_raw, _trace=False, **bkw):
    import ml_dtypes
    from concourse.bass_utils import run_bass_kernel_spmd
    if _trace:
        _trace = _ensure_trace_hook()

    bf16 = ml_dtypes.bfloat16
    x = np.asarray(x, f32)
    w_raw = np.asarray(w_raw, f32)
    B, L, C_in, d = x.shape
    N = B * L
    w = np.exp((w_raw - f32(np.log(C_in))).astype(f32)).astype(f32)
    w = (w / w.sum(axis=0, keepdims=True)).astype(f32)

    xr = x.reshape(N, C_in, d)
    # per core: [NBLK, 2, R, i, d] -> transpose to [NBLK, 2, i, R, d]
    xcore = xr.reshape(N_CORES, NBLK, 2, R, C_in, d)
    xp = np.ascontiguousarray(xcore.transpose(0, 1, 2, 4, 3, 5)).astype(bf16)
    x0p = np.ascontiguousarray(xcore[:, :, :, :, 0, :]).astype(bf16)
    w_rep = np.ascontiguousarray(
        np.broadcast_to(w.T.reshape(1, 64, 1, 64), (2, 64, R, 64))
        .transpose(0, 3, 2, 1).reshape(128, W)).astype(bf16)
    # w_rep[p, (j, o)]: lower/upper halves identical, = w[i=p%64, o]
    w_rep = np.ascontiguousarray(
        np.tile(np.repeat(w[None, :, :], 1, axis=0), (2, 1, 1))  # (2,64,64)
        .reshape(2, 64, 1, 64).repeat(R, axis=2).reshape(2 * 64, R * 64)
        ).astype(bf16)
    ident2 = np.tile(np.eye(64, dtype=bf16), (2, 1))

    nc = _get_program(**bkw)
    in_maps = []
    for k in range(N_CORES):
        in_maps.append({
            "xp": xp[k],
            "x0p": x0p[k],
            "w_rep": w_rep,
            "ident2": ident2,
        })
    res = run_bass_kernel_spmd(nc, in_maps, core_ids=list(range(N_CORES)),
                               trace=_trace)
    # out_p: [NBLK, 2, o, j, d] per core -> rows
    outs = []
    for k in range(N_CORES):
        op = res.results[k]["out_p"]          # (NBLK, 2, 64, R, 64)
        outs.append(np.ascontiguousarray(op.transpose(0, 1, 3, 2, 4))
                    .reshape(ROWS_PER_CORE, C_OUT, d))
    out = np.concatenate(outs, axis=0)
    if _trace:
        kernel.last_exec_time_ns = res.exec_time_ns
        kernel.last_results = res
    return out.reshape(B, L, C_OUT, d).astype(f32)
